# revision 1
# baseline (speedup 1.0000x reference)
"""AFT-Full (Attention Free Transformer, full position bias) on 8 TRN2
NeuronCores.

Problem (per reference.py):
    x [16, 2048, 512] f32, Wq/Wk/Wv [512, 512], bq/bk/bv [512],
    pos_bias [2048, 2048]
    q = x@Wq+bq; k = x@Wk+bk; v = x@Wv+bv
    out[b,i,d] = sigmoid(q)[b,i,d]
                 * sum_j exp(k+bias[i,j])*v / sum_j exp(k+bias[i,j])

Sharding: pure data-parallel over the batch (16 batches -> 2 per core).
Every core holds a replica of the weights and exp(pos_bias^T); there is
zero cross-core communication, so each core runs one self-contained
kernel on its batch shard and the host concatenates the 8 outputs.

Host-side prep is layout only (transpose / concat / shard); all compute
(projections, exp, the two [N,N]x[N,D] contractions, the sigmoid gate)
runs on-device.  The max-subtraction in the reference cancels exactly in
num/den, so the kernel exponentiates raw values (ranges here are small).

Math notes used by the device kernel:
  - bk cancels exactly:  exp(k+bk) appears in num and den with the same
    per-d factor exp(bk), so the unbiased k gives the same output.
    (The kernel still handles nonzero biases via an extra K=1 matmul
    row when any bias is nonzero; with the reference's zero biases the
    fast path compiles without it.)
  - sigmoid(q)*num/den = num*exp(q) / (den*(1+exp(q))), so the scalar
    engine only ever evaluates Exp (a single activation table set).

Compute dtype: bf16 on the TensorEngine with f32 PSUM accumulation
(fp8 was measured to inject ~3.6% output error here -- the output is a
near-cancelling weighted mean of zero-mean v, so weight-quantization
noise passes through at full strength -- while bf16 lands at ~0.3%).
"""

from contextlib import ExitStack

import numpy as np

import concourse.bacc as bacc
import concourse.mybir as mybir
import concourse.tile as tile
from concourse.bass_utils import run_bass_kernel_spmd

F32 = mybir.dt.float32
BF16 = mybir.dt.bfloat16
F8 = mybir.dt.float8e4
P = 128

N_CORES = 8
BATCH = 16
N = 2048
D_MODEL = 512


def _install_axon_ntff_shim():
    """Make run_bass_kernel_spmd(trace=True) work when the image's antenv
    lacks axon_hooks (the hook degrades tracing otherwise).  No-op when a
    real antenv.axon_hooks is importable."""
    import sys
    import types

    try:
        import antenv.axon_hooks  # noqa: F401
        return
    except ImportError:
        pass
    try:
        from trn_agent_boot.trn_boot import _ntff_profile_via_ctypes
        hook = _ntff_profile_via_ctypes("/opt/axon/libaxon_pjrt.so")
    except Exception:
        hook = None
    mod = types.ModuleType("antenv.axon_hooks")
    mod.get_axon_ntff_profile_hook = lambda: hook
    mod.set_axon_ntff_profile_hook = lambda h: None
    sys.modules["antenv.axon_hooks"] = mod

    import concourse.bass_utils as bass_utils
    _orig_upload = bass_utils.upload_artifacts

    def _safe_upload(tmpdir):
        try:
            return _orig_upload(tmpdir)
        except Exception:
            return tmpdir

    bass_utils.upload_artifacts = _safe_upload


def build_aft(B=2, N=2048, D=512, n_cores=8, use_bias=False, fp8_s2=False):
    NT = N // P          # row tiles per batch (t / j / i tiles)
    DB = D // P          # d_model blocks of 128 (contraction for projections)
    QKV = 3 * D
    C2 = 2 * B * D       # stage-2 psum width: [num_b0|den_b0|num_b1|den_b1]
    XW = 4 * P           # x DMA batching: four t-tiles per transfer (2KB runs)
    S2DT = F8 if fp8_s2 else BF16

    nc = bacc.Bacc("TRN2", target_bir_lowering=False, debug=False,
                   num_devices=n_cores)

    xT_e = nc.dram_tensor("xT", [B, D, N], F32, kind="ExternalInput")
    w_e = nc.dram_tensor("wvkq", [D, QKV], F32, kind="ExternalInput")
    pbT_e = nc.dram_tensor("pbT", [N, N], F32, kind="ExternalInput")
    if use_bias:
        b_e = nc.dram_tensor("bvkq", [1, QKV], F32, kind="ExternalInput")
    out_e = nc.dram_tensor("out", [B, N, D], F32, kind="ExternalOutput")

    with tile.TileContext(nc) as tc, ExitStack() as ctx:
        persist = ctx.enter_context(tc.tile_pool(name="persist", bufs=1))
        psp = ctx.enter_context(tc.tile_pool(name="psum", bufs=2, space="PSUM"))

        # ---- persistent SBUF tensors ----
        ebT_sb = persist.tile([P, NT, N], S2DT)          # exp(pos_bias)^T
        ekv_sb = persist.tile([P, NT, 2 * B * D], S2DT)  # [ev|ek] per batch
        q_sb = persist.tile([P, B * NT, D], BF16)        # exp(q)

        with ExitStack() as s1:
            wpool = s1.enter_context(tc.tile_pool(name="wpool", bufs=1))
            stage = s1.enter_context(tc.tile_pool(name="stage", bufs=2))
            xstage = s1.enter_context(tc.tile_pool(name="xstage", bufs=3))

            # ---- PE warmup ----
            # The PE clock-gate (HAM) starts at 1.2GHz and releases to
            # 2.4GHz only after ~3.4us of sustained activity.  The first
            # ~14us of the kernel are DMA-bound with an idle PE, so issue
            # throwaway matmuls on a memset tile to warm the clock before
            # the first real projection arrives.
            wa = wpool.tile([P, 64], BF16)
            nc.gpsimd.memset(wa[:], 0.0)
            wps = psp.tile([P, C2], F32, tag="ps", name="wps")
            for w_i in range(125):
                nc.tensor.matmul(wps[0:64, 0:64], wa[:, 0:64], wa[:],
                                 start=(w_i == 0), stop=(w_i == 124))

            # ---- weights: DMA f32 per d-block + cast to bf16 ----
            # db0 is split [v | kq] so the very first matmul's weights (v,
            # db0) arrive with a minimal transfer instead of waiting for
            # 3MB of weight DMA to drain.
            w_sb = wpool.tile([P, DB, QKV], BF16)        # rhs for projections
            w_r = w_e.ap().rearrange("(db p) c -> db p c", p=P)
            IOW = max(N // 2, QKV)
            w_st = stage.tile([P, IOW], F32, tag="io", name="w_st")
            nc.sync.dma_start(w_st[:, :D], w_r[0][:, :D])
            nc.vector.tensor_copy(w_sb[:, 0, :D], w_st[:, :D])
            w_st = stage.tile([P, IOW], F32, tag="io", name="w_st")
            nc.sync.dma_start(w_st[:, :QKV - D], w_r[0][:, D:])
            nc.vector.tensor_copy(w_sb[:, 0, D:], w_st[:, :QKV - D])
            for db in range(1, DB):
                w_st = stage.tile([P, IOW], F32, tag="io", name="w_st")
                nc.sync.dma_start(w_st[:, :QKV], w_r[db])
                nc.vector.tensor_copy(w_sb[:, db, :], w_st[:, :QKV])
            if use_bias:
                b_st = stage.tile([1, QKV], F32, tag="bst")
                nc.sync.dma_start(b_st[:], b_e.ap())
                bias_sb = wpool.tile([1, QKV], BF16)
                nc.vector.tensor_copy(bias_sb[:], b_st[:])
                ones_sb = wpool.tile([1, P], BF16)
                nc.vector.memset(ones_sb[:], 1.0)

            # ---- stage 1: projections v/k/q + exp epilogue ----
            # pos-bias blocks are paced into the loop (left column half
            # only, one 512KB block per two t-tiles) so the pbT stream
            # doesn't starve the x DMAs feeding the projections.
            xT_r = xT_e.ap().rearrange("b (db p) n -> b p db n", p=P)
            XT = XW // P         # t-tiles per x transfer
            # chunk the t axis: small leading chunks on batch 0 so the first
            # matmul's x tile doesn't wait behind a 2MB transfer
            def x_chunks(b, NT=NT, XT=XT):
                sizes = [1, 1, 1, 1] if b == 0 else []
                while sum(sizes) < NT:
                    sizes.append(min(XT, NT - sum(sizes)))
                return sizes

            # flatten (b, t) and emit each tile's x cast one iteration
            # AHEAD of the previous tile's epilogue: in the DVE FIFO the
            # cast then sits before ev/q (which wait on ACT's exp(k)), so
            # the next projections never starve on a cast that is ready
            chunk_info = []          # (b, t0, cw) per transfer
            tile_chunk = {}          # global tile s -> (chunk idx, tloc)
            for b in range(B):
                t0 = 0
                for cw in x_chunks(b):
                    for u in range(cw):
                        tile_chunk[b * NT + t0 + u] = (len(chunk_info), u)
                    chunk_info.append((b, t0, cw))
                    t0 += cw

            x_st_t = {}              # chunk idx -> staged tile
            x_bf_t = {}              # global tile s -> bf16 tile

            def emit_chunk_dma(cidx):
                cb, ct0, cw = chunk_info[cidx]
                x_st = xstage.tile([P, DB, XW], F32, tag="xst", bufs=2,
                                   name="x_st")
                nc.sync.dma_start(
                    x_st[:, :, :cw * P],
                    xT_r[cb, :, :, ct0 * P:(ct0 + cw) * P])
                x_st_t[cidx] = x_st

            def emit_cast(s):
                cidx, tloc = tile_chunk[s]
                x_bf = xstage.tile([P, DB, P], BF16, tag="xbf",
                                   name="x_bf")
                nc.vector.tensor_copy(
                    x_bf[:], x_st_t[cidx][:, :, tloc * P:(tloc + 1) * P])
                x_bf_t[s] = x_bf

            emit_chunk_dma(0)
            emit_cast(0)
            step = 0
            for b in range(B):
                for t in range(NT):
                    s = b * NT + t
                    if s + 1 < B * NT:
                        c_next, tloc_next = tile_chunk[s + 1]
                        if tloc_next == 0:
                            emit_chunk_dma(c_next)
                        emit_cast(s + 1)
                    x_bf = x_bf_t.pop(s)

                    ps = psp.tile([P, C2], F32, tag="ps")
                    for db in range(DB):
                        for n3 in range(3):   # [v|k|q]
                            nc.tensor.matmul(
                                ps[:, n3 * D:(n3 + 1) * D],
                                x_bf[:, db, :],
                                w_sb[:, db, n3 * D:(n3 + 1) * D],
                                start=(db == 0),
                                stop=(db == DB - 1 and not use_bias))
                    if use_bias:
                        for n3 in range(3):
                            nc.tensor.matmul(
                                ps[:, n3 * D:(n3 + 1) * D],
                                ones_sb[:, :],
                                bias_sb[:, n3 * D:(n3 + 1) * D],
                                start=False, stop=True)

                    col = b * 2 * D
                    # ek = exp(k)
                    nc.scalar.activation(ekv_sb[:, t, col + D:col + 2 * D],
                                         ps[:, D:2 * D],
                                         mybir.ActivationFunctionType.Exp)
                    # ev = ek * v
                    nc.vector.tensor_mul(ekv_sb[:, t, col:col + D],
                                         ekv_sb[:, t, col + D:col + 2 * D],
                                         ps[:, 0:D])
                    # raw q -> bf16 on DVE; exp(q) runs on the idle stage-2
                    # ACT so stage-1 ACT stays off the PSUM critical path
                    nc.vector.tensor_copy(q_sb[:, b * NT + t, :],
                                          ps[:, 2 * D:3 * D])

                    # pace pos-bias blocks, LEFT COLUMN HALF only: stage-2
                    # i-tile i reads columns i*128:(i+1)*128 of each block,
                    # so the right half isn't needed until i=NT/2 -- it is
                    # loaded during stage 2 where DMA is otherwise idle.
                    # One 512KB half-block per two t-tiles covers all NT
                    # blocks within stage 1 without crowding the x DMAs.
                    if step >= 7 and step % 2 == 1:
                        jb = (step - 7) // 2
                        if jb < NT:
                            pb_st = stage.tile([P, IOW], F32, tag="io",
                                               name="pb_st")
                            nc.sync.dma_start(
                                pb_st[:, :N // 2],
                                pbT_e.ap()[jb * P:(jb + 1) * P, :N // 2])
                            nc.scalar.activation(
                                ebT_sb[:, jb, :N // 2], pb_st[:, :N // 2],
                                mybir.ActivationFunctionType.Exp)
                    step += 1

            # left-half remainder: 512KB blocks load fast at the stage-2
            # head and are consumed last by i=0's ascending jb sweep
            for jb in range(max(0, (step - 7 + 1) // 2), NT):
                pb_st = stage.tile([P, IOW], F32, tag="io", name="pb_st")
                nc.sync.dma_start(pb_st[:, :N // 2],
                                  pbT_e.ap()[jb * P:(jb + 1) * P, :N // 2])
                nc.scalar.activation(ebT_sb[:, jb, :N // 2],
                                     pb_st[:, :N // 2],
                                     mybir.ActivationFunctionType.Exp)

        # ---- stage 2: num/den contraction over j + epilogue ----
        epi = ctx.enter_context(tc.tile_pool(name="epi", bufs=3))
        pb1p = ctx.enter_context(tc.tile_pool(name="pb1p", bufs=2))
        DR = mybir.MatmulPerfMode.DoubleRow

        # exp(q) in place over the raw-q buffer, on the otherwise-idle
        # stage-2 ACT (emitted after the spilled pos-bias exps above).
        # Batch-interleaved order (t, NT+t, t+1, ...) so i=0's epilogue
        # finds both of its eq tiles near the front of the ACT queue
        # instead of waiting for tile NT at position NT+1.
        for t_i in range(NT):
            for b_i in range(B):
                tt = b_i * NT + t_i
                nc.scalar.activation(q_sb[:, tt, :], q_sb[:, tt, :],
                                     mybir.ActivationFunctionType.Exp)

        for i in range(NT):
            ps = psp.tile([P, C2], F32, tag="ps")
            if fp8_s2:
                for jb2 in range(NT // 2):
                    lhsT = ebT_sb[:, 2 * jb2:2 * jb2 + 2, i * P:(i + 1) * P]
                    for n4 in range(2 * B):
                        nc.tensor.matmul(
                            ps[:, n4 * D:(n4 + 1) * D],
                            lhsT,
                            ekv_sb[:, 2 * jb2:2 * jb2 + 2,
                                   n4 * D:(n4 + 1) * D],
                            start=(jb2 == 0), stop=(jb2 == NT // 2 - 1),
                            perf_mode=DR)
            else:
                for jb in range(NT):
                    lhsT = ebT_sb[:, jb, i * P:(i + 1) * P]
                    for n4 in range(2 * B):
                        nc.tensor.matmul(
                            ps[:, n4 * D:(n4 + 1) * D],
                            lhsT,
                            ekv_sb[:, jb, n4 * D:(n4 + 1) * D],
                            start=(jb == 0), stop=(jb == NT - 1))

            # right column halves of exp(pos_bias^T): two blocks per
            # i-tile while i < NT/2, finishing just before i = NT/2 reads
            # them; DMA and ACT are both idle in this window
            if i < NT // 2:
                for u in range(2):
                    jbr = 2 * i + u
                    pb1 = pb1p.tile([P, N // 2], F32, tag="pb1")
                    nc.sync.dma_start(
                        pb1[:], pbT_e.ap()[jbr * P:(jbr + 1) * P, N // 2:])
                    nc.scalar.activation(ebT_sb[:, jbr, N // 2:], pb1[:],
                                         mybir.ActivationFunctionType.Exp)

            o = epi.tile([P, B, D], F32, tag="o")
            for b in range(B):
                nu = ps[:, b * 2 * D:b * 2 * D + D]
                de = ps[:, b * 2 * D + D:b * 2 * D + 2 * D]
                eq = q_sb[:, b * NT + i, :]
                # t1 = (exp(q) + 1) * den
                t1 = epi.tile([P, D], F32, tag="t1")
                nc.vector.scalar_tensor_tensor(
                    t1[:], eq, 1.0, de,
                    mybir.AluOpType.add, mybir.AluOpType.mult)
                r = epi.tile([P, D], F32, tag="r")
                nc.vector.reciprocal_approx_fast(r[:], t1[:])
                # o = num * exp(q) * r
                o1 = epi.tile([P, D], F32, tag="o1")
                nc.vector.tensor_mul(o1[:], nu, eq)
                nc.vector.tensor_mul(o[:, b, :], o1[:], r[:])
                if i == NT - 1:
                    # last tile: per-batch DMA so the b0 store overlaps the
                    # b1 epilogue instead of extending the kernel tail
                    nc.sync.dma_start(out_e.ap()[b, i * P:(i + 1) * P],
                                      o[:, b, :])
            if i < NT - 1:
                nc.sync.dma_start(
                    out_e.ap().rearrange("b n d -> n b d")[i * P:(i + 1) * P],
                    o[:])

    nc.compile()
    return nc


_NC_CACHE = {}


def _get_nc(use_bias):
    key = bool(use_bias)
    if key not in _NC_CACHE:
        _NC_CACHE[key] = build_aft(B=BATCH // N_CORES, N=N, D=D_MODEL,
                                   n_cores=N_CORES, use_bias=key)
    return _NC_CACHE[key]


def kernel(x, Wq, bq, Wk, bk, Wv, bv, pos_bias):
    x = np.asarray(x, dtype=np.float32)
    Wq = np.asarray(Wq, dtype=np.float32)
    Wk = np.asarray(Wk, dtype=np.float32)
    Wv = np.asarray(Wv, dtype=np.float32)
    bq = np.asarray(bq, dtype=np.float32)
    bk = np.asarray(bk, dtype=np.float32)
    bv = np.asarray(bv, dtype=np.float32)
    pos_bias = np.asarray(pos_bias, dtype=np.float32)
    assert x.shape == (BATCH, N, D_MODEL)
    assert pos_bias.shape == (N, N)

    _install_axon_ntff_shim()

    use_bias = bool(np.any(bq) or np.any(bk) or np.any(bv))
    nc = _get_nc(use_bias)

    Bc = BATCH // N_CORES
    wvkq = np.concatenate([Wv, Wk, Wq], axis=1)           # [D, 3D]
    pbT = np.ascontiguousarray(pos_bias.T)                # [N, N]
    in_maps = []
    for c in range(N_CORES):
        im = {
            "xT": np.ascontiguousarray(
                x[c * Bc:(c + 1) * Bc].transpose(0, 2, 1)),
            "wvkq": wvkq,
            "pbT": pbT,
        }
        if use_bias:
            im["bvkq"] = np.concatenate([bv, bk, bq])[None, :]
        in_maps.append(im)

    res = run_bass_kernel_spmd(nc, in_maps, core_ids=list(range(N_CORES)))
    out = np.concatenate([res.results[c]["out"] for c in range(N_CORES)],
                         axis=0)
    return out.astype(np.float32, copy=False)



# revision 14
# speedup vs baseline: 1.1544x; 1.1544x over previous
"""AFT-Full (Attention Free Transformer, full position bias) on 8 TRN2
NeuronCores.

Problem (per reference.py):
    x [16, 2048, 512] f32, Wq/Wk/Wv [512, 512], bq/bk/bv [512],
    pos_bias [2048, 2048]
    q = x@Wq+bq; k = x@Wk+bk; v = x@Wv+bv
    out[b,i,d] = sigmoid(q)[b,i,d]
                 * sum_j exp(k+bias[i,j])*v / sum_j exp(k+bias[i,j])

Sharding: pure data-parallel over the batch (16 batches -> 2 per core).
Every core holds a replica of the weights and pos_bias; there is zero
cross-core communication.

Numerics / speed strategy:
  - Stage 1 (projections v/k/q) runs in bf16 on the TensorEngine.
  - Stage 2 (the [N,N] x [N,2BD] num/den contraction) runs in fp8e4
    with the DoubleRow perf mode (2 contraction rows per PE pass).
    Naive fp8 here costs ~3.6% output error because the output is a
    near-cancelling weighted mean of zero-mean v and per-term
    quantization noise passes straight through.  Instead we use the
    shifted decomposition
        eb = exp(pos_bias) = 1 + u,   u = exp(pos_bias) - 1
        num[i,d] = sum_j ev[j,d]  +  sum_j u[i,j] ev[j,d]
        den[i,d] = sum_j ek[j,d]  +  sum_j u[i,j] ek[j,d]
    The first (i-independent) colsum terms carry ~90% of the magnitude
    and are accumulated exactly in f32 on the sbuf side; only the small
    u-contraction runs in fp8 (u has RMS ~0.1 vs eb ~1.0), cutting the
    fp8 noise by ~10x (to ~0.4% total, vs the 2e-2 harness gate).
    u is scaled by 512 and [ev|ek] by 8 to keep fp8 values in the
    normal e4m3 range; the epilogue divides the PSUM result by 4096
    and adds the colsums back before the sigmoid gate.
  - sigmoid(q)*num/den = num*exp(q) / (den*(1+exp(q))), so the scalar
    engine only ever evaluates Exp.
"""

from contextlib import ExitStack

import numpy as np

import concourse.bacc as bacc
import concourse.mybir as mybir
import concourse.tile as tile
from concourse.bass_isa import ReduceOp
from concourse.bass_utils import run_bass_kernel_spmd

F32 = mybir.dt.float32
BF16 = mybir.dt.bfloat16
F8 = mybir.dt.float8e4
P = 128

N_CORES = 8
BATCH = 16
N = 2048
D_MODEL = 512

# mybir float8e4 is IEEE-style e4m3: max finite 240, overflow -> inf.
# Keep scaled maxima comfortably below 240 (|u|<~0.75, |ev|<~30, ek<~12).
U_SCALE = 256.0     # fp8 scale for u = exp(pos_bias) - 1
KV_SCALE = 4.0      # fp8 scale for [ev|ek]
INV_SCALE = 1.0 / (U_SCALE * KV_SCALE)


def _install_axon_ntff_shim():
    """Make run_bass_kernel_spmd(trace=True) work when the image's antenv
    lacks axon_hooks (the hook degrades tracing otherwise).  No-op when a
    real antenv.axon_hooks is importable."""
    import sys
    import types

    try:
        import antenv.axon_hooks  # noqa: F401
        return
    except ImportError:
        pass
    try:
        from trn_agent_boot.trn_boot import _ntff_profile_via_ctypes
        hook = _ntff_profile_via_ctypes("/opt/axon/libaxon_pjrt.so")
    except Exception:
        hook = None
    mod = types.ModuleType("antenv.axon_hooks")
    mod.get_axon_ntff_profile_hook = lambda: hook
    mod.set_axon_ntff_profile_hook = lambda h: None
    sys.modules["antenv.axon_hooks"] = mod

    import concourse.bass_utils as bass_utils
    _orig_upload = bass_utils.upload_artifacts

    def _safe_upload(tmpdir):
        try:
            return _orig_upload(tmpdir)
        except Exception:
            return tmpdir

    bass_utils.upload_artifacts = _safe_upload


def build_aft(B=2, N=2048, D=512, n_cores=8, use_bias=False):
    NT = N // P          # row tiles per batch (t / j / i tiles)
    DB = D // P          # d_model blocks of 128 (contraction for projections)
    QKV = 3 * D
    C2 = 2 * B * D       # stage-2 psum width: [num_b0|den_b0|num_b1|den_b1]
    XW = 4 * P           # x DMA batching: four t-tiles per transfer (2KB runs)
    Exp = mybir.ActivationFunctionType.Exp
    Alu = mybir.AluOpType
    DR = mybir.MatmulPerfMode.DoubleRow
    F32R = mybir.dt.float32r

    nc = bacc.Bacc("TRN2", target_bir_lowering=False, debug=False,
                   num_devices=n_cores)

    xT_e = nc.dram_tensor("xT", [B, D, N], F32, kind="ExternalInput")
    w_e = nc.dram_tensor("wvkq", [D, QKV], F32, kind="ExternalInput")
    pbT_e = nc.dram_tensor("pbT", [N, N], F32, kind="ExternalInput")
    if use_bias:
        b_e = nc.dram_tensor("bvkq", [1, QKV], F32, kind="ExternalInput")
    out_e = nc.dram_tensor("out", [B, N, D], F32, kind="ExternalOutput")

    with tile.TileContext(nc) as tc, ExitStack() as ctx:
        persist = ctx.enter_context(tc.tile_pool(name="persist", bufs=1))
        psp = ctx.enter_context(tc.tile_pool(name="psum", bufs=2, space="PSUM"))

        # ---- persistent SBUF tensors ----
        u8_sb = persist.tile([P, NT, N], F8)             # 512*(exp(pbT)-1)
        ekv_sb = persist.tile([P, NT, 2 * B * D], F8)    # 8*[ev|ek] per batch
        q_sb = persist.tile([P, B * NT, D], BF16)        # exp(q)
        acc_sb = persist.tile([P, 2 * B * D], F32)       # colsum accumulator
        cs_sb = persist.tile([P, 2 * B * D], F32)        # all-reduced colsums

        with ExitStack() as s1:
            wpool = s1.enter_context(tc.tile_pool(name="wpool", bufs=1))
            stage = s1.enter_context(tc.tile_pool(name="stage", bufs=2))
            xstage = s1.enter_context(tc.tile_pool(name="xstage", bufs=3))
            kvpool = s1.enter_context(tc.tile_pool(name="kvpool", bufs=3))
            ebpool = s1.enter_context(tc.tile_pool(name="ebpool", bufs=2))

            # ---- PE warmup ----
            # The PE clock-gate (HAM) starts at 1.2GHz and releases to
            # 2.4GHz only after ~3.4us of sustained activity.  The first
            # ~14us of the kernel are DMA-bound with an idle PE, so issue
            # throwaway matmuls on a memset tile to warm the clock before
            # the first real projection arrives.
            wa = wpool.tile([P, 64], BF16)
            nc.gpsimd.memset(wa[:], 0.0)
            nc.gpsimd.memset(acc_sb[:], 0.0)
            wps = psp.tile([P, C2], F32, tag="ps", name="wps")
            for w_i in range(125):
                nc.tensor.matmul(wps[0:64, 0:64], wa[:, 0:64], wa[:],
                                 start=(w_i == 0), stop=(w_i == 124))

            # ---- weights: DMA f32 per d-block + cast to bf16 ----
            # db0 is split [v | kq] so the very first matmul's weights (v,
            # db0) arrive with a minimal transfer instead of waiting for
            # 3MB of weight DMA to drain.
            w_sb = wpool.tile([P, DB, QKV], BF16)        # rhs for projections
            w_r = w_e.ap().rearrange("(db p) c -> db p c", p=P)
            IOW = max(N // 2, QKV)
            w_st = stage.tile([P, IOW], F32, tag="io", name="w_st")
            nc.sync.dma_start(w_st[:, :D], w_r[0][:, :D])
            nc.vector.tensor_copy(w_sb[:, 0, :D], w_st[:, :D])
            w_st = stage.tile([P, IOW], F32, tag="io", name="w_st")
            nc.sync.dma_start(w_st[:, :QKV - D], w_r[0][:, D:])
            nc.vector.tensor_copy(w_sb[:, 0, D:], w_st[:, :QKV - D])
            for db in range(1, DB):
                w_st = stage.tile([P, IOW], F32, tag="io", name="w_st")
                nc.sync.dma_start(w_st[:, :QKV], w_r[db])
                nc.vector.tensor_copy(w_sb[:, db, :], w_st[:, :QKV])
            if use_bias:
                b_st = stage.tile([1, QKV], F32, tag="bst")
                nc.sync.dma_start(b_st[:], b_e.ap())
                bias_sb = wpool.tile([1, QKV], BF16)
                nc.vector.tensor_copy(bias_sb[:], b_st[:])
                ones_sb = wpool.tile([1, P], BF16)
                nc.vector.memset(ones_sb[:], 1.0)

            # ---- stage 1: projections v/k/q + exp epilogue ----
            # pos-bias blocks are paced into the loop (left column half
            # only, one 512KB block per two t-tiles) so the pbT stream
            # doesn't starve the x DMAs feeding the projections.
            xT_r = xT_e.ap().rearrange("b (db p) n -> b p db n", p=P)
            XT = XW // P         # t-tiles per x transfer
            # chunk the t axis: small leading chunks on batch 0 so the first
            # matmul's x tile doesn't wait behind a 2MB transfer
            def x_chunks(b, NT=NT, XT=XT):
                sizes = [1, 1, 1, 1] if b == 0 else []
                while sum(sizes) < NT:
                    sizes.append(min(XT, NT - sum(sizes)))
                return sizes

            # flatten (b, t) and emit each tile's x cast one iteration
            # AHEAD of the previous tile's epilogue: in the DVE FIFO the
            # cast then sits before ev (which waits on ACT's exp(k)), so
            # the next projections never starve on a cast that is ready
            chunk_info = []          # (b, t0, cw) per transfer
            tile_chunk = {}          # global tile s -> (chunk idx, tloc)
            for b in range(B):
                t0 = 0
                for cw in x_chunks(b):
                    for u in range(cw):
                        tile_chunk[b * NT + t0 + u] = (len(chunk_info), u)
                    chunk_info.append((b, t0, cw))
                    t0 += cw

            x_st_t = {}              # chunk idx -> staged tile
            x_bf_t = {}              # global tile s -> bf16 tile

            def emit_chunk_dma(cidx):
                cb, ct0, cw = chunk_info[cidx]
                x_st = xstage.tile([P, DB, XW], F32, tag="xst", bufs=2,
                                   name="x_st")
                nc.sync.dma_start(
                    x_st[:, :, :cw * P],
                    xT_r[cb, :, :, ct0 * P:(ct0 + cw) * P])
                x_st_t[cidx] = x_st

            def emit_cast(s):
                cidx, tloc = tile_chunk[s]
                x_bf = xstage.tile([P, DB, P], BF16, tag="xbf",
                                   name="x_bf")
                nc.vector.tensor_copy(
                    x_bf[:], x_st_t[cidx][:, :, tloc * P:(tloc + 1) * P])
                x_bf_t[s] = x_bf

            def emit_pb_left(jb):
                # left column half of the shifted pos-bias transform:
                # u8 = (exp(pbT) - 1) * 512 in fp8
                pb_st = stage.tile([P, IOW], F32, tag="io", name="pb_st")
                nc.sync.dma_start(pb_st[:, :N // 2],
                                  pbT_e.ap()[jb * P:(jb + 1) * P, :N // 2])
                ebf = ebpool.tile([P, N // 2], BF16, tag="ebf")
                nc.scalar.activation(ebf[:], pb_st[:, :N // 2], Exp)
                nc.vector.tensor_scalar(u8_sb[:, jb, :N // 2], ebf[:],
                                        1.0, U_SCALE,
                                        Alu.subtract, Alu.mult)

            emit_chunk_dma(0)
            emit_cast(0)
            step = 0
            for b in range(B):
                for t in range(NT):
                    s = b * NT + t
                    if s + 1 < B * NT:
                        c_next, tloc_next = tile_chunk[s + 1]
                        if tloc_next == 0:
                            emit_chunk_dma(c_next)
                        emit_cast(s + 1)
                    x_bf = x_bf_t.pop(s)

                    ps = psp.tile([P, C2], F32, tag="ps")
                    for db in range(DB):
                        for n3 in range(3):   # [v|k|q]
                            nc.tensor.matmul(
                                ps[:, n3 * D:(n3 + 1) * D],
                                x_bf[:, db, :],
                                w_sb[:, db, n3 * D:(n3 + 1) * D],
                                start=(db == 0),
                                stop=(db == DB - 1 and not use_bias))
                    if use_bias:
                        for n3 in range(3):
                            nc.tensor.matmul(
                                ps[:, n3 * D:(n3 + 1) * D],
                                ones_sb[:, :],
                                bias_sb[:, n3 * D:(n3 + 1) * D],
                                start=False, stop=True)

                    col = b * 2 * D
                    # kv = [ev | ek] in bf16 (feeds the f32 colsum -- bf16
                    # element rounding costs only ~0.1% there -- and the
                    # fp8 cast; 16-bit keeps the DVE/gpsimd ops at 2x rate)
                    kv = kvpool.tile([P, 2 * D], BF16, tag="kv")
                    nc.scalar.activation(kv[:, D:2 * D], ps[:, D:2 * D], Exp)
                    nc.vector.tensor_mul(kv[:, 0:D], kv[:, D:2 * D],
                                         ps[:, 0:D])
                    # exp(q) straight from PSUM on the scalar engine
                    nc.scalar.activation(q_sb[:, b * NT + t, :],
                                         ps[:, 2 * D:3 * D], Exp)
                    # fp8 cast (scaled) in one DVE op
                    nc.vector.tensor_scalar_mul(
                        ekv_sb[:, t, col:col + 2 * D], kv[:], KV_SCALE)
                    # f32 colsum accumulation on gpsimd
                    nc.gpsimd.tensor_add(acc_sb[:, col:col + 2 * D],
                                         acc_sb[:, col:col + 2 * D], kv[:])

                    # pace pos-bias blocks, LEFT COLUMN HALF only: stage-2
                    # i-tile i reads columns i*128:(i+1)*128 of each block,
                    # so the right half isn't needed until i=NT/2 -- it is
                    # loaded during stage 2 where DMA is otherwise idle.
                    # One 512KB half-block per two t-tiles covers all NT
                    # blocks within stage 1 without crowding the x DMAs.
                    if step >= 7 and step % 2 == 1:
                        jb = (step - 7) // 2
                        if jb < NT:
                            emit_pb_left(jb)
                    step += 1

                # batch b's colsum: reduce acc over partitions and
                # broadcast to all partitions, on the (otherwise idle)
                # gpsimd engine.  b0's runs overlapped with b1's stage 1.
                col = b * 2 * D
                nc.gpsimd.partition_all_reduce(
                    cs_sb[:, col:col + 2 * D], acc_sb[:, col:col + 2 * D],
                    P, ReduceOp.add)

            # left-half remainder: 512KB blocks load fast at the stage-2
            # head and are consumed last by i=0's ascending jb sweep
            for jb in range(max(0, (step - 7 + 1) // 2), NT):
                emit_pb_left(jb)

        # ---- stage 2: num/den contraction over j + epilogue ----
        epi = ctx.enter_context(tc.tile_pool(name="epi", bufs=3))
        pb1p = ctx.enter_context(tc.tile_pool(name="pb1p", bufs=2))

        for i in range(NT):
            ps = psp.tile([P, C2], F32, tag="ps")
            for jb2 in range(NT // 2):
                lhsT = u8_sb[:, 2 * jb2:2 * jb2 + 2, i * P:(i + 1) * P]
                for n4 in range(2 * B):
                    nc.tensor.matmul(
                        ps[:, n4 * D:(n4 + 1) * D],
                        lhsT,
                        ekv_sb[:, 2 * jb2:2 * jb2 + 2,
                               n4 * D:(n4 + 1) * D],
                        start=(jb2 == 0), stop=(jb2 == NT // 2 - 1),
                        perf_mode=DR)

            # right column halves of the shifted pos-bias: two blocks per
            # i-tile while i < NT/2, finishing just before i = NT/2 reads
            # them; DMA and ACT are both idle in this window
            if i < NT // 2:
                for u in range(2):
                    jbr = 2 * i + u
                    pb1 = pb1p.tile([P, N // 2], F32, tag="pb1")
                    nc.sync.dma_start(
                        pb1[:], pbT_e.ap()[jbr * P:(jbr + 1) * P, N // 2:])
                    eb1 = pb1p.tile([P, N // 2], BF16, tag="eb1")
                    nc.scalar.activation(eb1[:], pb1[:], Exp)
                    nc.vector.tensor_scalar(u8_sb[:, jbr, N // 2:], eb1[:],
                                            1.0, U_SCALE,
                                            Alu.subtract, Alu.mult)

            o = epi.tile([P, B, D], F32, tag="o")
            for b in range(B):
                col = b * 2 * D
                nu = ps[:, col:col + D]
                de = ps[:, col + D:col + 2 * D]
                eq = q_sb[:, b * NT + i, :]
                # num/den = psum/4096 + colsum   (the exact shifted term)
                num = epi.tile([P, D], F32, tag="num")
                nc.vector.scalar_tensor_tensor(
                    num[:], nu, INV_SCALE, cs_sb[:, col:col + D],
                    Alu.mult, Alu.add)
                den = epi.tile([P, D], F32, tag="den")
                nc.vector.scalar_tensor_tensor(
                    den[:], de, INV_SCALE, cs_sb[:, col + D:col + 2 * D],
                    Alu.mult, Alu.add)
                # t1 = (exp(q) + 1) * den
                t1 = epi.tile([P, D], F32, tag="t1")
                nc.vector.scalar_tensor_tensor(
                    t1[:], eq, 1.0, den[:], Alu.add, Alu.mult)
                r = epi.tile([P, D], F32, tag="r")
                nc.vector.reciprocal_approx_fast(r[:], t1[:])
                # o = num * exp(q) * r
                o1 = epi.tile([P, D], F32, tag="o1")
                nc.vector.tensor_mul(o1[:], num[:], eq)
                nc.vector.tensor_mul(o[:, b, :], o1[:], r[:])
                if i == NT - 1:
                    # last tile: per-batch DMA so the b0 store overlaps the
                    # b1 epilogue instead of extending the kernel tail
                    nc.sync.dma_start(out_e.ap()[b, i * P:(i + 1) * P],
                                      o[:, b, :])
            if i < NT - 1:
                nc.sync.dma_start(
                    out_e.ap().rearrange("b n d -> n b d")[i * P:(i + 1) * P],
                    o[:])

    nc.compile()
    return nc


_NC_CACHE = {}


def _get_nc(use_bias):
    key = bool(use_bias)
    if key not in _NC_CACHE:
        _NC_CACHE[key] = build_aft(B=BATCH // N_CORES, N=N, D=D_MODEL,
                                   n_cores=N_CORES, use_bias=key)
    return _NC_CACHE[key]


def kernel(x, Wq, bq, Wk, bk, Wv, bv, pos_bias):
    x = np.asarray(x, dtype=np.float32)
    Wq = np.asarray(Wq, dtype=np.float32)
    Wk = np.asarray(Wk, dtype=np.float32)
    Wv = np.asarray(Wv, dtype=np.float32)
    bq = np.asarray(bq, dtype=np.float32)
    bk = np.asarray(bk, dtype=np.float32)
    bv = np.asarray(bv, dtype=np.float32)
    pos_bias = np.asarray(pos_bias, dtype=np.float32)
    assert x.shape == (BATCH, N, D_MODEL)
    assert pos_bias.shape == (N, N)

    _install_axon_ntff_shim()

    use_bias = bool(np.any(bq) or np.any(bk) or np.any(bv))
    nc = _get_nc(use_bias)

    Bc = BATCH // N_CORES
    wvkq = np.concatenate([Wv, Wk, Wq], axis=1)           # [D, 3D]
    pbT = np.ascontiguousarray(pos_bias.T)                # [N, N]
    in_maps = []
    for c in range(N_CORES):
        im = {
            "xT": np.ascontiguousarray(
                x[c * Bc:(c + 1) * Bc].transpose(0, 2, 1)),
            "wvkq": wvkq,
            "pbT": pbT,
        }
        if use_bias:
            im["bvkq"] = np.concatenate([bv, bk, bq])[None, :]
        in_maps.append(im)

    res = run_bass_kernel_spmd(nc, in_maps, core_ids=list(range(N_CORES)))
    out = np.concatenate([res.results[c]["out"] for c in range(N_CORES)],
                         axis=0)
    return out.astype(np.float32, copy=False)


# revision 26
# speedup vs baseline: 1.2331x; 1.0681x over previous
"""AFT-Full (Attention Free Transformer, full position bias) on 8 TRN2
NeuronCores.

Problem (per reference.py):
    x [16, 2048, 512] f32, Wq/Wk/Wv [512, 512], bq/bk/bv [512],
    pos_bias [2048, 2048]
    q = x@Wq+bq; k = x@Wk+bk; v = x@Wv+bv
    out[b,i,d] = sigmoid(q)[b,i,d]
                 * sum_j exp(k+bias[i,j])*v / sum_j exp(k+bias[i,j])

Sharding: pure data-parallel over the batch (16 batches -> 2 per core).
Every core holds a replica of the weights and pos_bias; there is zero
cross-core communication.

Numerics / speed strategy:
  - Stage 1 (projections v/k/q) runs in bf16 on the TensorEngine.
  - Stage 2 (the [N,N] x [N,2BD] num/den contraction) runs in fp8e4
    with the DoubleRow perf mode (2 contraction rows per PE pass).
    Naive fp8 here costs ~3.6% output error because the output is a
    near-cancelling weighted mean of zero-mean v and per-term
    quantization noise passes straight through.  Instead we use the
    shifted decomposition
        eb = exp(pos_bias) = 1 + u,   u = exp(pos_bias) - 1
        num[i,d] = sum_j ev[j,d]  +  sum_j u[i,j] ev[j,d]
        den[i,d] = sum_j ek[j,d]  +  sum_j u[i,j] ek[j,d]
    The first (i-independent) colsum terms carry ~90% of the magnitude
    and are accumulated exactly in f32 on the sbuf side; only the small
    u-contraction runs in fp8 (u has RMS ~0.1 vs eb ~1.0), cutting the
    fp8 noise by ~10x (to ~0.4% total, vs the 2e-2 harness gate).
    u is scaled by 512 and [ev|ek] by 8 to keep fp8 values in the
    normal e4m3 range; the epilogue divides the PSUM result by 4096
    and adds the colsums back before the sigmoid gate.
  - sigmoid(q)*num/den = num*exp(q) / (den*(1+exp(q))), so the scalar
    engine only ever evaluates Exp.
"""

from contextlib import ExitStack

import numpy as np

import concourse.bacc as bacc
import concourse.mybir as mybir
import concourse.tile as tile
from concourse.bass_isa import ReduceOp
from concourse.bass_utils import run_bass_kernel_spmd

F32 = mybir.dt.float32
BF16 = mybir.dt.bfloat16
F8 = mybir.dt.float8e4
P = 128

N_CORES = 8
BATCH = 16
N = 2048
D_MODEL = 512

# mybir float8e4 is IEEE-style e4m3: max finite 240, overflow -> inf.
# Keep scaled maxima comfortably below 240 (|u|<~0.75, |ev|<~30, ek<~12).
U_SCALE = 256.0     # fp8 scale for u = exp(pos_bias) - 1
KV_SCALE = 4.0      # fp8 scale for [ev|ek]
INV_SCALE = 1.0 / (U_SCALE * KV_SCALE)


def _install_axon_ntff_shim():
    """Make run_bass_kernel_spmd(trace=True) work when the image's antenv
    lacks axon_hooks (the hook degrades tracing otherwise).  No-op when a
    real antenv.axon_hooks is importable."""
    import sys
    import types

    try:
        import antenv.axon_hooks  # noqa: F401
        return
    except ImportError:
        pass
    try:
        from trn_agent_boot.trn_boot import _ntff_profile_via_ctypes
        hook = _ntff_profile_via_ctypes("/opt/axon/libaxon_pjrt.so")
    except Exception:
        hook = None
    mod = types.ModuleType("antenv.axon_hooks")
    mod.get_axon_ntff_profile_hook = lambda: hook
    mod.set_axon_ntff_profile_hook = lambda h: None
    sys.modules["antenv.axon_hooks"] = mod

    import concourse.bass_utils as bass_utils
    _orig_upload = bass_utils.upload_artifacts

    def _safe_upload(tmpdir):
        try:
            return _orig_upload(tmpdir)
        except Exception:
            return tmpdir

    bass_utils.upload_artifacts = _safe_upload


def build_aft(B=2, N=2048, D=512, n_cores=8, use_bias=False):
    NT = N // P          # row tiles per batch (t / j / i tiles)
    DB = D // P          # d_model blocks of 128 (contraction for projections)
    QKV = 3 * D
    C2 = 2 * B * D       # stage-2 psum width: [num_b0|den_b0|num_b1|den_b1]
    XW = 4 * P           # x DMA batching: four t-tiles per transfer (2KB runs)
    Exp = mybir.ActivationFunctionType.Exp
    Ident = mybir.ActivationFunctionType.Identity
    Alu = mybir.AluOpType
    DR = mybir.MatmulPerfMode.DoubleRow
    F32R = mybir.dt.float32r

    nc = bacc.Bacc("TRN2", target_bir_lowering=False, debug=False,
                   num_devices=n_cores)

    xT_e = nc.dram_tensor("xT", [B, D, N], F32, kind="ExternalInput")
    w_e = nc.dram_tensor("wvkq", [D, QKV], F32, kind="ExternalInput")
    pbT_e = nc.dram_tensor("pbT", [N, N], F32, kind="ExternalInput")
    if use_bias:
        b_e = nc.dram_tensor("bvkq", [1, QKV], F32, kind="ExternalInput")
    out_e = nc.dram_tensor("out", [B, N, D], F32, kind="ExternalOutput")

    with tile.TileContext(nc) as tc, ExitStack() as ctx:
        persist = ctx.enter_context(tc.tile_pool(name="persist", bufs=1))
        psp = ctx.enter_context(tc.tile_pool(name="psum", bufs=2, space="PSUM"))

        # ---- persistent SBUF tensors ----
        u8_sb = persist.tile([P, NT, N], F8)             # 512*(exp(pbT)-1)
        ekv_sb = persist.tile([P, NT, 2 * B * D], F8)    # 8*[ev|ek] per batch
        q_sb = persist.tile([P, B * NT, D], BF16)        # exp(q)
        acc_sb = persist.tile([P, 2 * B * D], F32)       # colsum accumulator
        cs_sb = persist.tile([P, 2 * B * D], F32)        # all-reduced colsums
        cneg = persist.tile([P, 1], F32)                 # bias AP: -U_SCALE
        nc.gpsimd.memset(cneg[:], -U_SCALE)

        with ExitStack() as s1:
            wpool = s1.enter_context(tc.tile_pool(name="wpool", bufs=1))
            stage = s1.enter_context(tc.tile_pool(name="stage", bufs=2))
            xstage = s1.enter_context(tc.tile_pool(name="xstage", bufs=3))
            kvpool = s1.enter_context(tc.tile_pool(name="kvpool", bufs=3))
            ebpool = s1.enter_context(tc.tile_pool(name="ebpool", bufs=2))

            # ---- PE warmup ----
            # The PE clock-gate (HAM) starts at 1.2GHz and releases to
            # 2.4GHz only after ~3.4us of sustained activity.  The first
            # ~14us of the kernel are DMA-bound with an idle PE, so issue
            # throwaway matmuls on a memset tile to warm the clock before
            # the first real projection arrives.
            wa = wpool.tile([P, 64], BF16)
            nc.gpsimd.memset(wa[:], 0.0)
            nc.gpsimd.memset(acc_sb[:], 0.0)
            wps = psp.tile([P, C2], F32, tag="ps", name="wps")
            for w_i in range(125):
                nc.tensor.matmul(wps[0:64, 0:64], wa[:, 0:64], wa[:],
                                 start=(w_i == 0), stop=(w_i == 124))

            # ---- weights: DMA f32 per d-block + cast to bf16 ----
            # db0 is split [v | kq] so the very first matmul's weights (v,
            # db0) arrive with a minimal transfer instead of waiting for
            # 3MB of weight DMA to drain.
            w_sb = wpool.tile([P, DB, QKV], BF16)        # rhs for projections
            w_r = w_e.ap().rearrange("(db p) c -> db p c", p=P)
            IOW = max(N // 2, QKV)
            w_st = stage.tile([P, IOW], F32, tag="io", name="w_st")
            nc.sync.dma_start(w_st[:, :D], w_r[0][:, :D])
            nc.vector.tensor_copy(w_sb[:, 0, :D], w_st[:, :D])
            w_st = stage.tile([P, IOW], F32, tag="io", name="w_st")
            nc.sync.dma_start(w_st[:, :QKV - D], w_r[0][:, D:])
            nc.vector.tensor_copy(w_sb[:, 0, D:], w_st[:, :QKV - D])
            for db in range(1, DB):
                w_st = stage.tile([P, IOW], F32, tag="io", name="w_st")
                nc.sync.dma_start(w_st[:, :QKV], w_r[db])
                nc.vector.tensor_copy(w_sb[:, db, :], w_st[:, :QKV])
            if use_bias:
                b_st = stage.tile([1, QKV], F32, tag="bst")
                nc.sync.dma_start(b_st[:], b_e.ap())
                bias_sb = wpool.tile([1, QKV], BF16)
                nc.vector.tensor_copy(bias_sb[:], b_st[:])
                ones_sb = wpool.tile([1, P], BF16)
                nc.vector.memset(ones_sb[:], 1.0)

            # ---- stage 1: projections v/k/q + exp epilogue ----
            # pos-bias blocks are paced into the loop (left column half
            # only, one 512KB block per two t-tiles) so the pbT stream
            # doesn't starve the x DMAs feeding the projections.
            xT_r = xT_e.ap().rearrange("b (db p) n -> b p db n", p=P)
            XT = XW // P         # t-tiles per x transfer
            # chunk the t axis: small leading chunks on batch 0 so the first
            # matmul's x tile doesn't wait behind a 2MB transfer
            def x_chunks(b, NT=NT, XT=XT):
                sizes = [1, 1, 1, 1] if b == 0 else []
                while sum(sizes) < NT:
                    sizes.append(min(XT, NT - sum(sizes)))
                return sizes

            # x is cast bf16 at CHUNK granularity, one contiguous DVE op
            # per transfer (a per-tile strided cast measured ~2x slower
            # per element); the cast is emitted right after the chunk DMA
            # so it runs while the previous chunk's matmuls execute.
            chunk_info = []          # (b, t0, cw) per transfer
            tile_chunk = {}          # global tile s -> (chunk idx, tloc)
            for b in range(B):
                t0 = 0
                for cw in x_chunks(b):
                    for u in range(cw):
                        tile_chunk[b * NT + t0 + u] = (len(chunk_info), u)
                    chunk_info.append((b, t0, cw))
                    t0 += cw

            x_bf_c = {}              # chunk idx -> bf16 chunk tile

            def emit_chunk_dma(cidx):
                cb, ct0, cw = chunk_info[cidx]
                x_st = xstage.tile([P, DB, XW], F32, tag="xst", bufs=2,
                                   name="x_st")
                nc.sync.dma_start(
                    x_st[:, :, :cw * P],
                    xT_r[cb, :, :, ct0 * P:(ct0 + cw) * P])
                x_bf = xstage.tile([P, DB, XW], BF16, tag="xbf", bufs=2,
                                   name="x_bf")
                nc.vector.tensor_copy(x_bf[:, :, :cw * P],
                                      x_st[:, :, :cw * P])
                x_bf_c[cidx] = x_bf

            def emit_pb_left(jb):
                # left column half of the shifted pos-bias transform:
                # u8 = exp(pbT)*256 - 256 in fp8, both steps on the scalar
                # engine (keeps the DVE free for the x casts / ekv)
                pb_st = stage.tile([P, IOW], F32, tag="io", name="pb_st")
                nc.sync.dma_start(pb_st[:, :N // 2],
                                  pbT_e.ap()[jb * P:(jb + 1) * P, :N // 2])
                ebf = ebpool.tile([P, N // 2], F32, tag="ebf")
                nc.scalar.activation(ebf[:], pb_st[:, :N // 2], Exp)
                nc.scalar.activation(u8_sb[:, jb, :N // 2], ebf[:], Ident,
                                     bias=cneg[:], scale=U_SCALE)

            emit_chunk_dma(0)
            step = 0
            for b in range(B):
                for t in range(NT):
                    s = b * NT + t
                    if s + 1 < B * NT:
                        c_next, tloc_next = tile_chunk[s + 1]
                        if tloc_next == 0:
                            emit_chunk_dma(c_next)
                    cidx, tloc = tile_chunk[s]
                    x_bf = x_bf_c[cidx]

                    ps = psp.tile([P, C2], F32, tag="ps")
                    for db in range(DB):
                        for n3 in range(3):   # [v|k|q]
                            nc.tensor.matmul(
                                ps[:, n3 * D:(n3 + 1) * D],
                                x_bf[:, db, tloc * P:(tloc + 1) * P],
                                w_sb[:, db, n3 * D:(n3 + 1) * D],
                                start=(db == 0),
                                stop=(db == DB - 1 and not use_bias))
                    if use_bias:
                        for n3 in range(3):
                            nc.tensor.matmul(
                                ps[:, n3 * D:(n3 + 1) * D],
                                ones_sb[:, :],
                                bias_sb[:, n3 * D:(n3 + 1) * D],
                                start=False, stop=True)

                    col = b * 2 * D
                    # kv = [ev | ek] in bf16 (feeds the f32 colsum -- bf16
                    # element rounding costs only ~0.1% there -- and the
                    # fp8 cast; 16-bit keeps the DVE/gpsimd ops at 2x rate)
                    kv = kvpool.tile([P, 2 * D], BF16, tag="kv")
                    nc.scalar.activation(kv[:, D:2 * D], ps[:, D:2 * D], Exp)
                    nc.vector.tensor_mul(kv[:, 0:D], kv[:, D:2 * D],
                                         ps[:, 0:D])
                    # em = exp(-q) straight from PSUM on the scalar engine;
                    # the epilogue uses sigmoid(q)*num/den =
                    # num / (den * (1 + exp(-q)))
                    nc.scalar.activation(q_sb[:, b * NT + t, :],
                                         ps[:, 2 * D:3 * D], Exp, scale=-1.0)
                    # fp8 cast (scaled) in one DVE op
                    nc.vector.tensor_scalar_mul(
                        ekv_sb[:, t, col:col + 2 * D], kv[:], KV_SCALE)
                    # f32 colsum accumulation on gpsimd
                    nc.gpsimd.tensor_add(acc_sb[:, col:col + 2 * D],
                                         acc_sb[:, col:col + 2 * D], kv[:])

                    # pace pos-bias blocks, LEFT COLUMN HALF only: stage-2
                    # i-tile i reads columns i*128:(i+1)*128 of each block,
                    # so the right half isn't needed until i=NT/2 -- it is
                    # loaded during stage 2 where DMA is otherwise idle.
                    # One 512KB half-block per two t-tiles covers all NT
                    # blocks within stage 1 without crowding the x DMAs.
                    if step >= 7 and step % 2 == 1:
                        jb = (step - 7) // 2
                        if jb < NT:
                            emit_pb_left(jb)
                    step += 1

                # batch b's colsum: reduce acc over partitions and
                # broadcast to all partitions, on the (otherwise idle)
                # gpsimd engine.  b0's runs overlapped with b1's stage 1.
                col = b * 2 * D
                nc.gpsimd.partition_all_reduce(
                    cs_sb[:, col:col + 2 * D], acc_sb[:, col:col + 2 * D],
                    P, ReduceOp.add)

            # left-half remainder: 512KB blocks load fast at the stage-2
            # head and are consumed last by i=0's ascending jb sweep
            for jb in range(max(0, (step - 7 + 1) // 2), NT):
                emit_pb_left(jb)

        # ---- stage 2: num/den contraction over j + epilogue ----
        epi = ctx.enter_context(tc.tile_pool(name="epi", bufs=3))
        pb1p = ctx.enter_context(tc.tile_pool(name="pb1p", bufs=2))

        # emp1 = 1 + exp(-q), in place over q_sb on the stage-2 scalar
        # engine (idle at this point); batch-interleaved so i=0's two
        # tiles are at the front of the queue
        for t_i in range(NT):
            for b_i in range(B):
                tt = b_i * NT + t_i
                nc.scalar.activation(q_sb[:, tt, :], q_sb[:, tt, :], Ident,
                                     bias=1.0)

        for i in range(NT):
            ps = psp.tile([P, C2], F32, tag="ps")
            for jb2 in range(NT // 2):
                lhsT = u8_sb[:, 2 * jb2:2 * jb2 + 2, i * P:(i + 1) * P]
                for n4 in range(2 * B):
                    nc.tensor.matmul(
                        ps[:, n4 * D:(n4 + 1) * D],
                        lhsT,
                        ekv_sb[:, 2 * jb2:2 * jb2 + 2,
                               n4 * D:(n4 + 1) * D],
                        start=(jb2 == 0), stop=(jb2 == NT // 2 - 1),
                        perf_mode=DR)

            # right column halves of the shifted pos-bias: two blocks per
            # i-tile while i < NT/2, finishing just before i = NT/2 reads
            # them; DMA and ACT are both idle in this window
            if i < NT // 2:
                for u in range(2):
                    jbr = 2 * i + u
                    pb1 = pb1p.tile([P, N // 2], F32, tag="pb1")
                    nc.sync.dma_start(
                        pb1[:], pbT_e.ap()[jbr * P:(jbr + 1) * P, N // 2:])
                    eb1 = pb1p.tile([P, N // 2], F32, tag="eb1")
                    nc.scalar.activation(eb1[:], pb1[:], Exp)
                    nc.scalar.activation(u8_sb[:, jbr, N // 2:], eb1[:],
                                         Ident, bias=cneg[:], scale=U_SCALE)

            o = epi.tile([P, B, D], F32, tag="o")
            for b in range(B):
                col = b * 2 * D
                nu = ps[:, col:col + D]
                de = ps[:, col + D:col + 2 * D]
                emp1 = q_sb[:, b * NT + i, :]
                # num/den = psum/1024 + colsum   (the exact shifted term)
                num = epi.tile([P, D], F32, tag="num")
                nc.vector.scalar_tensor_tensor(
                    num[:], nu, INV_SCALE, cs_sb[:, col:col + D],
                    Alu.mult, Alu.add)
                den = epi.tile([P, D], F32, tag="den")
                nc.vector.scalar_tensor_tensor(
                    den[:], de, INV_SCALE, cs_sb[:, col + D:col + 2 * D],
                    Alu.mult, Alu.add)
                # t1 = (1 + exp(-q)) * den, a plain multiply on the idle
                # stage-2 gpsimd (Pool supports tensor_tensor only)
                t1 = epi.tile([P, D], F32, tag="t1")
                nc.gpsimd.tensor_mul(t1[:], emp1, den[:])
                r = epi.tile([P, D], F32, tag="r")
                nc.vector.reciprocal_approx_fast(r[:], t1[:])
                nc.vector.tensor_mul(o[:, b, :], num[:], r[:])
                if i == NT - 1:
                    # last tile: per-batch DMA so the b0 store overlaps the
                    # b1 epilogue instead of extending the kernel tail
                    nc.sync.dma_start(out_e.ap()[b, i * P:(i + 1) * P],
                                      o[:, b, :])
            if i < NT - 1:
                nc.sync.dma_start(
                    out_e.ap().rearrange("b n d -> n b d")[i * P:(i + 1) * P],
                    o[:])

    nc.compile()
    return nc


_NC_CACHE = {}


def _get_nc(use_bias):
    key = bool(use_bias)
    if key not in _NC_CACHE:
        _NC_CACHE[key] = build_aft(B=BATCH // N_CORES, N=N, D=D_MODEL,
                                   n_cores=N_CORES, use_bias=key)
    return _NC_CACHE[key]


def kernel(x, Wq, bq, Wk, bk, Wv, bv, pos_bias):
    x = np.asarray(x, dtype=np.float32)
    Wq = np.asarray(Wq, dtype=np.float32)
    Wk = np.asarray(Wk, dtype=np.float32)
    Wv = np.asarray(Wv, dtype=np.float32)
    bq = np.asarray(bq, dtype=np.float32)
    bk = np.asarray(bk, dtype=np.float32)
    bv = np.asarray(bv, dtype=np.float32)
    pos_bias = np.asarray(pos_bias, dtype=np.float32)
    assert x.shape == (BATCH, N, D_MODEL)
    assert pos_bias.shape == (N, N)

    _install_axon_ntff_shim()

    use_bias = bool(np.any(bq) or np.any(bk) or np.any(bv))
    nc = _get_nc(use_bias)

    Bc = BATCH // N_CORES
    wvkq = np.concatenate([Wv, Wk, Wq], axis=1)           # [D, 3D]
    pbT = np.ascontiguousarray(pos_bias.T)                # [N, N]
    in_maps = []
    for c in range(N_CORES):
        im = {
            "xT": np.ascontiguousarray(
                x[c * Bc:(c + 1) * Bc].transpose(0, 2, 1)),
            "wvkq": wvkq,
            "pbT": pbT,
        }
        if use_bias:
            im["bvkq"] = np.concatenate([bv, bk, bq])[None, :]
        in_maps.append(im)

    res = run_bass_kernel_spmd(nc, in_maps, core_ids=list(range(N_CORES)))
    out = np.concatenate([res.results[c]["out"] for c in range(N_CORES)],
                         axis=0)
    return out.astype(np.float32, copy=False)


# revision 31
# speedup vs baseline: 1.3221x; 1.0722x over previous
"""AFT-Full (Attention Free Transformer, full position bias) on 8 TRN2
NeuronCores.

Problem (per reference.py):
    x [16, 2048, 512] f32, Wq/Wk/Wv [512, 512], bq/bk/bv [512],
    pos_bias [2048, 2048]
    q = x@Wq+bq; k = x@Wk+bk; v = x@Wv+bv
    out[b,i,d] = sigmoid(q)[b,i,d]
                 * sum_j exp(k+bias[i,j])*v / sum_j exp(k+bias[i,j])

Sharding: pure data-parallel over the batch (16 batches -> 2 per core).
Every core holds a replica of the weights and pos_bias; there is zero
cross-core communication.

Numerics / speed strategy:
  - Stage 1 (projections v/k/q) runs in bf16 on the TensorEngine.
  - Stage 2 (the [N,N] x [N,2BD] num/den contraction) runs in fp8e4
    with the DoubleRow perf mode (2 contraction rows per PE pass).
    Naive fp8 here costs ~3.6% output error because the output is a
    near-cancelling weighted mean of zero-mean v and per-term
    quantization noise passes straight through.  Instead we use the
    shifted decomposition
        eb = exp(pos_bias) = 1 + u,   u = exp(pos_bias) - 1
        num[i,d] = sum_j ev[j,d]  +  sum_j u[i,j] ev[j,d]
        den[i,d] = sum_j ek[j,d]  +  sum_j u[i,j] ek[j,d]
    The first (i-independent) colsum terms carry ~90% of the magnitude
    and are accumulated exactly in f32 on the sbuf side; only the small
    u-contraction runs in fp8 (u has RMS ~0.1 vs eb ~1.0), cutting the
    fp8 noise by ~10x (to ~0.4% total, vs the 2e-2 harness gate).
    u is scaled by 512 and [ev|ek] by 8 to keep fp8 values in the
    normal e4m3 range; the epilogue divides the PSUM result by 4096
    and adds the colsums back before the sigmoid gate.
  - sigmoid(q)*num/den = num*exp(q) / (den*(1+exp(q))), so the scalar
    engine only ever evaluates Exp.
"""

from contextlib import ExitStack

import numpy as np

import concourse.bacc as bacc
import concourse.mybir as mybir
import concourse.tile as tile
from concourse.bass_isa import ReduceOp
from concourse.bass_utils import run_bass_kernel_spmd

F32 = mybir.dt.float32
BF16 = mybir.dt.bfloat16
F8 = mybir.dt.float8e4
P = 128

N_CORES = 8
BATCH = 16
N = 2048
D_MODEL = 512

# mybir float8e4 is IEEE-style e4m3: max finite 240, overflow -> inf.
# Keep scaled maxima comfortably below 240 (|u|<~0.75, |ev|<~30, ek<~12).
U_SCALE = 256.0     # fp8 scale for u = exp(pos_bias) - 1
KV_SCALE = 4.0      # fp8 scale for [ev|ek]
INV_SCALE = 1.0 / (U_SCALE * KV_SCALE)


def _install_axon_ntff_shim():
    """Make run_bass_kernel_spmd(trace=True) work when the image's antenv
    lacks axon_hooks (the hook degrades tracing otherwise).  No-op when a
    real antenv.axon_hooks is importable."""
    import sys
    import types

    try:
        import antenv.axon_hooks  # noqa: F401
        return
    except ImportError:
        pass
    try:
        from trn_agent_boot.trn_boot import _ntff_profile_via_ctypes
        hook = _ntff_profile_via_ctypes("/opt/axon/libaxon_pjrt.so")
    except Exception:
        hook = None
    mod = types.ModuleType("antenv.axon_hooks")
    mod.get_axon_ntff_profile_hook = lambda: hook
    mod.set_axon_ntff_profile_hook = lambda h: None
    sys.modules["antenv.axon_hooks"] = mod

    import concourse.bass_utils as bass_utils
    _orig_upload = bass_utils.upload_artifacts

    def _safe_upload(tmpdir):
        try:
            return _orig_upload(tmpdir)
        except Exception:
            return tmpdir

    bass_utils.upload_artifacts = _safe_upload


def build_aft(B=2, N=2048, D=512, n_cores=8, use_bias=False):
    NT = N // P          # row tiles per batch (t / j / i tiles)
    DB = D // P          # d_model blocks of 128 (contraction for projections)
    QKV = 3 * D
    C2 = 2 * B * D       # stage-2 psum width: [num_b0|den_b0|num_b1|den_b1]
    XW = 4 * P           # x DMA batching: four t-tiles per transfer (2KB runs)
    Exp = mybir.ActivationFunctionType.Exp
    Ident = mybir.ActivationFunctionType.Identity
    Alu = mybir.AluOpType
    DR = mybir.MatmulPerfMode.DoubleRow
    F32R = mybir.dt.float32r

    nc = bacc.Bacc("TRN2", target_bir_lowering=False, debug=False,
                   num_devices=n_cores)

    xT_e = nc.dram_tensor("xT", [B, D, N], F32, kind="ExternalInput")
    w_e = nc.dram_tensor("wvkq", [D, QKV], F32, kind="ExternalInput")
    pbT_e = nc.dram_tensor("pbT", [N, N], F32, kind="ExternalInput")
    if use_bias:
        b_e = nc.dram_tensor("bvkq", [1, QKV], F32, kind="ExternalInput")
    out_e = nc.dram_tensor("out", [B, N, D], F32, kind="ExternalOutput")

    with tile.TileContext(nc) as tc, ExitStack() as ctx:
        persist = ctx.enter_context(tc.tile_pool(name="persist", bufs=1))
        psp = ctx.enter_context(tc.tile_pool(name="psum", bufs=2, space="PSUM"))

        # ---- persistent SBUF tensors ----
        u8_sb = persist.tile([P, NT, N], F8)             # 512*(exp(pbT)-1)
        ekv_sb = persist.tile([P, NT, 2 * B * D], F8)    # 8*[ev|ek] per batch
        q_sb = persist.tile([P, B * NT, D], BF16)        # exp(q)
        acc_sb = persist.tile([P, 2 * B * D], F32)       # colsum accumulator
        cs_sb = persist.tile([P, 2 * B * D], F32)        # all-reduced colsums
        cneg = persist.tile([P, 1], F32)                 # bias AP: -U_SCALE
        nc.gpsimd.memset(cneg[:], -U_SCALE)

        with ExitStack() as s1:
            wpool = s1.enter_context(tc.tile_pool(name="wpool", bufs=1))
            stage = s1.enter_context(tc.tile_pool(name="stage", bufs=2))
            xstage = s1.enter_context(tc.tile_pool(name="xstage", bufs=2))
            # deep kv pool: the colsum-accumulate chains drain a few tiles
            # behind the ACT/DVE epilogue ops and must not backpressure them
            kvpool = s1.enter_context(tc.tile_pool(name="kvpool", bufs=6))
            ebpool = s1.enter_context(tc.tile_pool(name="ebpool", bufs=2))

            # ---- PE warmup ----
            # The PE clock-gate (HAM) starts at 1.2GHz and releases to
            # 2.4GHz only after ~3.4us of sustained activity.  The first
            # ~14us of the kernel are DMA-bound with an idle PE, so issue
            # throwaway matmuls on a memset tile to warm the clock before
            # the first real projection arrives.
            wa = wpool.tile([P, 64], BF16)
            nc.gpsimd.memset(wa[:], 0.0)
            nc.gpsimd.memset(acc_sb[:], 0.0)
            wps = psp.tile([P, C2], F32, tag="ps", name="wps")
            for w_i in range(125):
                nc.tensor.matmul(wps[0:64, 0:64], wa[:, 0:64], wa[:],
                                 start=(w_i == 0), stop=(w_i == 124))

            # ---- weights: DMA f32 per d-block + cast to bf16 ----
            # db0 is split [v | kq] so the very first matmul's weights (v,
            # db0) arrive with a minimal transfer instead of waiting for
            # 3MB of weight DMA to drain.
            w_sb = wpool.tile([P, DB, QKV], BF16)        # rhs for projections
            w_r = w_e.ap().rearrange("(db p) c -> db p c", p=P)
            IOW = max(N // 2, QKV)
            w_st = stage.tile([P, IOW], F32, tag="io", name="w_st")
            nc.sync.dma_start(w_st[:, :D], w_r[0][:, :D])
            nc.vector.tensor_copy(w_sb[:, 0, :D], w_st[:, :D])
            w_st = stage.tile([P, IOW], F32, tag="io", name="w_st")
            nc.sync.dma_start(w_st[:, :QKV - D], w_r[0][:, D:])
            nc.vector.tensor_copy(w_sb[:, 0, D:], w_st[:, :QKV - D])
            for db in range(1, DB):
                w_st = stage.tile([P, IOW], F32, tag="io", name="w_st")
                nc.sync.dma_start(w_st[:, :QKV], w_r[db])
                nc.vector.tensor_copy(w_sb[:, db, :], w_st[:, :QKV])
            if use_bias:
                b_st = stage.tile([1, QKV], F32, tag="bst")
                nc.sync.dma_start(b_st[:], b_e.ap())
                bias_sb = wpool.tile([1, QKV], BF16)
                nc.vector.tensor_copy(bias_sb[:], b_st[:])
                ones_sb = wpool.tile([1, P], BF16)
                nc.vector.memset(ones_sb[:], 1.0)

            # ---- stage 1: projections v/k/q + exp epilogue ----
            # pos-bias blocks are paced into the loop (left column half
            # only, one 512KB block per two t-tiles) so the pbT stream
            # doesn't starve the x DMAs feeding the projections.
            xT_r = xT_e.ap().rearrange("b (db p) n -> b p db n", p=P)
            XT = XW // P         # t-tiles per x transfer
            # chunk the t axis: small leading chunks on batch 0 so the first
            # matmul's x tile doesn't wait behind a 2MB transfer
            def x_chunks(b, NT=NT, XT=XT):
                sizes = [1, 1, 1, 1] if b == 0 else []
                while sum(sizes) < NT:
                    sizes.append(min(XT, NT - sum(sizes)))
                return sizes

            # x is cast bf16 at CHUNK granularity, one contiguous DVE op
            # per transfer (a per-tile strided cast measured ~2x slower
            # per element); the cast is emitted right after the chunk DMA
            # so it runs while the previous chunk's matmuls execute.
            chunk_info = []          # (b, t0, cw) per transfer
            tile_chunk = {}          # global tile s -> (chunk idx, tloc)
            for b in range(B):
                t0 = 0
                for cw in x_chunks(b):
                    for u in range(cw):
                        tile_chunk[b * NT + t0 + u] = (len(chunk_info), u)
                    chunk_info.append((b, t0, cw))
                    t0 += cw

            x_bf_c = {}              # chunk idx -> bf16 chunk tile

            def emit_chunk_dma(cidx):
                cb, ct0, cw = chunk_info[cidx]
                x_st = xstage.tile([P, DB, XW], F32, tag="xst", bufs=2,
                                   name="x_st")
                nc.sync.dma_start(
                    x_st[:, :, :cw * P],
                    xT_r[cb, :, :, ct0 * P:(ct0 + cw) * P])
                x_bf = xstage.tile([P, DB, XW], BF16, tag="xbf", bufs=2,
                                   name="x_bf")
                nc.vector.tensor_copy(x_bf[:, :, :cw * P],
                                      x_st[:, :, :cw * P])
                x_bf_c[cidx] = x_bf

            def emit_pb_left(jb):
                # left column half of the shifted pos-bias transform:
                # u8 = exp(pbT)*256 - 256 in fp8, both steps on the scalar
                # engine (keeps the DVE free for the x casts / ekv)
                pb_st = stage.tile([P, IOW], F32, tag="io", name="pb_st")
                nc.sync.dma_start(pb_st[:, :N // 2],
                                  pbT_e.ap()[jb * P:(jb + 1) * P, :N // 2])
                ebf = ebpool.tile([P, N // 2], F32, tag="ebf")
                nc.scalar.activation(ebf[:], pb_st[:, :N // 2], Exp)
                nc.scalar.activation(u8_sb[:, jb, :N // 2], ebf[:], Ident,
                                     bias=cneg[:], scale=U_SCALE)

            emit_chunk_dma(0)
            step = 0
            for b in range(B):
                for t in range(NT):
                    s = b * NT + t
                    cidx, tloc = tile_chunk[s]
                    # prefetch the NEXT chunk at the FIRST tile of this one
                    # (a full chunk of matmuls of DMA lead time)
                    if tloc == 0 and cidx + 1 < len(chunk_info):
                        emit_chunk_dma(cidx + 1)
                    x_bf = x_bf_c[cidx]

                    ps = psp.tile([P, C2], F32, tag="ps")
                    for db in range(DB):
                        for n3 in range(3):   # [v|k|q]
                            nc.tensor.matmul(
                                ps[:, n3 * D:(n3 + 1) * D],
                                x_bf[:, db, tloc * P:(tloc + 1) * P],
                                w_sb[:, db, n3 * D:(n3 + 1) * D],
                                start=(db == 0),
                                stop=(db == DB - 1 and not use_bias))
                    if use_bias:
                        for n3 in range(3):
                            nc.tensor.matmul(
                                ps[:, n3 * D:(n3 + 1) * D],
                                ones_sb[:, :],
                                bias_sb[:, n3 * D:(n3 + 1) * D],
                                start=False, stop=True)

                    col = b * 2 * D
                    # kv = [ev | ek] in bf16 (feeds the f32 colsum -- bf16
                    # element rounding costs only ~0.1% there -- and the
                    # fp8 cast; 16-bit keeps the DVE/gpsimd ops at 2x rate)
                    kv = kvpool.tile([P, 2 * D], BF16, tag="kv")
                    nc.scalar.activation(kv[:, D:2 * D], ps[:, D:2 * D], Exp)
                    nc.vector.tensor_mul(kv[:, 0:D], kv[:, D:2 * D],
                                         ps[:, 0:D])
                    # em = exp(-q) straight from PSUM on the scalar engine;
                    # the epilogue uses sigmoid(q)*num/den =
                    # num / (den * (1 + exp(-q)))
                    nc.scalar.activation(q_sb[:, b * NT + t, :],
                                         ps[:, 2 * D:3 * D], Exp, scale=-1.0)
                    # fp8 cast (scaled) in one DVE op
                    nc.vector.tensor_scalar_mul(
                        ekv_sb[:, t, col:col + 2 * D], kv[:], KV_SCALE)
                    # f32 colsum accumulation: a serial per-batch chain.
                    # b0's chain runs on gpsimd; b1's runs on the (faster)
                    # DVE so it finishes with stage 1 and its all-reduce
                    # result is ready before stage-2 i=2 recycles PSUM.
                    acc_eng = nc.gpsimd if b == 0 else nc.vector
                    acc_eng.tensor_add(acc_sb[:, col:col + 2 * D],
                                       acc_sb[:, col:col + 2 * D], kv[:])

                    # pace pos-bias blocks, LEFT COLUMN HALF only: stage-2
                    # i-tile i reads columns i*128:(i+1)*128 of each block,
                    # so the right half isn't needed until i=NT/2 -- it is
                    # loaded during stage 2 where DMA is otherwise idle.
                    # One 512KB half-block per two t-tiles covers all NT
                    # blocks within stage 1 without crowding the x DMAs.
                    if step >= 7 and step % 2 == 1:
                        jb = (step - 7) // 2
                        if jb < NT:
                            emit_pb_left(jb)
                    step += 1

                # batch b's colsum: reduce acc over partitions and
                # broadcast to all partitions, on the (otherwise idle)
                # gpsimd engine.  b0's runs overlapped with b1's stage 1.
                col = b * 2 * D
                nc.gpsimd.partition_all_reduce(
                    cs_sb[:, col:col + 2 * D], acc_sb[:, col:col + 2 * D],
                    P, ReduceOp.add)

            # left-half remainder: 512KB blocks load fast at the stage-2
            # head and are consumed last by i=0's ascending jb sweep
            for jb in range(max(0, (step - 7 + 1) // 2), NT):
                emit_pb_left(jb)

        # ---- stage 2: num/den contraction over j + epilogue ----
        epi = ctx.enter_context(tc.tile_pool(name="epi", bufs=3))
        pb1p = ctx.enter_context(tc.tile_pool(name="pb1p", bufs=2))

        # right-half pos-bias pacing: 3 blocks per early i-tile so all 16
        # are transformed well before i = NT/2 reads them
        pbr_sched = {}
        jbr_next = 0
        for i in range(NT):
            take = min(3, NT - jbr_next)
            pbr_sched[i] = list(range(jbr_next, jbr_next + take))
            jbr_next += take

        for i in range(NT):
            ps = psp.tile([P, C2], F32, tag="ps")
            for jb2 in range(NT // 2):
                lhsT = u8_sb[:, 2 * jb2:2 * jb2 + 2, i * P:(i + 1) * P]
                for n4 in range(2 * B):
                    nc.tensor.matmul(
                        ps[:, n4 * D:(n4 + 1) * D],
                        lhsT,
                        ekv_sb[:, 2 * jb2:2 * jb2 + 2,
                               n4 * D:(n4 + 1) * D],
                        start=(jb2 == 0), stop=(jb2 == NT // 2 - 1),
                        perf_mode=DR)

            # right column halves of the shifted pos-bias: exp on the
            # scalar engine, the shift-and-quantize on the DVE (the ACT
            # is the scarcer engine here)
            for jbr in pbr_sched[i]:
                pb1 = pb1p.tile([P, N // 2], F32, tag="pb1")
                nc.sync.dma_start(
                    pb1[:], pbT_e.ap()[jbr * P:(jbr + 1) * P, N // 2:])
                eb1 = pb1p.tile([P, N // 2], BF16, tag="eb1")
                nc.scalar.activation(eb1[:], pb1[:], Exp)
                nc.vector.tensor_scalar(u8_sb[:, jbr, N // 2:], eb1[:],
                                        1.0, U_SCALE,
                                        Alu.subtract, Alu.mult)

            o = epi.tile([P, B, D], F32, tag="o")
            for b in range(B):
                col = b * 2 * D
                nu = ps[:, col:col + D]
                de = ps[:, col + D:col + 2 * D]
                emp1 = q_sb[:, b * NT + i, :]
                # emp1 = 1 + exp(-q), in place on the stage-2 scalar engine
                nc.scalar.activation(emp1, emp1, Ident, bias=1.0)
                # num/den = psum/1024 + colsum   (the exact shifted term)
                num = epi.tile([P, D], F32, tag="num")
                nc.vector.scalar_tensor_tensor(
                    num[:], nu, INV_SCALE, cs_sb[:, col:col + D],
                    Alu.mult, Alu.add)
                den = epi.tile([P, D], F32, tag="den")
                nc.vector.scalar_tensor_tensor(
                    den[:], de, INV_SCALE, cs_sb[:, col + D:col + 2 * D],
                    Alu.mult, Alu.add)
                # t1 = (1 + exp(-q)) * den, a plain multiply on the idle
                # stage-2 gpsimd (Pool supports tensor_tensor only); the
                # final i-tile keeps it on the DVE to shorten the tail
                t1 = epi.tile([P, D], F32, tag="t1")
                t1_eng = nc.vector if i == NT - 1 else nc.gpsimd
                t1_eng.tensor_mul(t1[:], emp1, den[:])
                r = epi.tile([P, D], F32, tag="r")
                nc.vector.reciprocal_approx_fast(r[:], t1[:])
                nc.vector.tensor_mul(o[:, b, :], num[:], r[:])
                if i == NT - 1:
                    # last tile: per-batch DMA so the b0 store overlaps the
                    # b1 epilogue instead of extending the kernel tail
                    nc.sync.dma_start(out_e.ap()[b, i * P:(i + 1) * P],
                                      o[:, b, :])
            if i < NT - 1:
                nc.sync.dma_start(
                    out_e.ap().rearrange("b n d -> n b d")[i * P:(i + 1) * P],
                    o[:])

    nc.compile()
    return nc


_NC_CACHE = {}


def _get_nc(use_bias):
    key = bool(use_bias)
    if key not in _NC_CACHE:
        _NC_CACHE[key] = build_aft(B=BATCH // N_CORES, N=N, D=D_MODEL,
                                   n_cores=N_CORES, use_bias=key)
    return _NC_CACHE[key]


def kernel(x, Wq, bq, Wk, bk, Wv, bv, pos_bias):
    x = np.asarray(x, dtype=np.float32)
    Wq = np.asarray(Wq, dtype=np.float32)
    Wk = np.asarray(Wk, dtype=np.float32)
    Wv = np.asarray(Wv, dtype=np.float32)
    bq = np.asarray(bq, dtype=np.float32)
    bk = np.asarray(bk, dtype=np.float32)
    bv = np.asarray(bv, dtype=np.float32)
    pos_bias = np.asarray(pos_bias, dtype=np.float32)
    assert x.shape == (BATCH, N, D_MODEL)
    assert pos_bias.shape == (N, N)

    _install_axon_ntff_shim()

    use_bias = bool(np.any(bq) or np.any(bk) or np.any(bv))
    nc = _get_nc(use_bias)

    Bc = BATCH // N_CORES
    wvkq = np.concatenate([Wv, Wk, Wq], axis=1)           # [D, 3D]
    pbT = np.ascontiguousarray(pos_bias.T)                # [N, N]
    in_maps = []
    for c in range(N_CORES):
        im = {
            "xT": np.ascontiguousarray(
                x[c * Bc:(c + 1) * Bc].transpose(0, 2, 1)),
            "wvkq": wvkq,
            "pbT": pbT,
        }
        if use_bias:
            im["bvkq"] = np.concatenate([bv, bk, bq])[None, :]
        in_maps.append(im)

    res = run_bass_kernel_spmd(nc, in_maps, core_ids=list(range(N_CORES)))
    out = np.concatenate([res.results[c]["out"] for c in range(N_CORES)],
                         axis=0)
    return out.astype(np.float32, copy=False)


# revision 34
# speedup vs baseline: 1.3583x; 1.0274x over previous
"""AFT-Full (Attention Free Transformer, full position bias) on 8 TRN2
NeuronCores.

Problem (per reference.py):
    x [16, 2048, 512] f32, Wq/Wk/Wv [512, 512], bq/bk/bv [512],
    pos_bias [2048, 2048]
    q = x@Wq+bq; k = x@Wk+bk; v = x@Wv+bv
    out[b,i,d] = sigmoid(q)[b,i,d]
                 * sum_j exp(k+bias[i,j])*v / sum_j exp(k+bias[i,j])

Sharding: pure data-parallel over the batch (16 batches -> 2 per core).
Every core holds a replica of the weights and pos_bias; there is zero
cross-core communication.

Numerics / speed strategy:
  - Stage 1 (projections v/k/q) runs in bf16 on the TensorEngine.
  - Stage 2 (the [N,N] x [N,2BD] num/den contraction) runs in fp8e4
    with the DoubleRow perf mode (2 contraction rows per PE pass).
    Naive fp8 here costs ~3.6% output error because the output is a
    near-cancelling weighted mean of zero-mean v and per-term
    quantization noise passes straight through.  Instead we use the
    shifted decomposition
        eb = exp(pos_bias) = 1 + u,   u = exp(pos_bias) - 1
        num[i,d] = sum_j ev[j,d]  +  sum_j u[i,j] ev[j,d]
        den[i,d] = sum_j ek[j,d]  +  sum_j u[i,j] ek[j,d]
    The first (i-independent) colsum terms carry ~90% of the magnitude
    and are accumulated exactly in f32 on the sbuf side; only the small
    u-contraction runs in fp8 (u has RMS ~0.1 vs eb ~1.0), cutting the
    fp8 noise by ~10x (to ~0.4% total, vs the 2e-2 harness gate).
    u is scaled by 512 and [ev|ek] by 8 to keep fp8 values in the
    normal e4m3 range; the epilogue divides the PSUM result by 4096
    and adds the colsums back before the sigmoid gate.
  - sigmoid(q)*num/den = num*exp(q) / (den*(1+exp(q))), so the scalar
    engine only ever evaluates Exp.
"""

from contextlib import ExitStack

import numpy as np

import concourse.bacc as bacc
import concourse.mybir as mybir
import concourse.tile as tile
from concourse.bass_isa import ReduceOp
from concourse.bass_utils import run_bass_kernel_spmd

F32 = mybir.dt.float32
BF16 = mybir.dt.bfloat16
F8 = mybir.dt.float8e4
P = 128

N_CORES = 8
BATCH = 16
N = 2048
D_MODEL = 512

# mybir float8e4 is IEEE-style e4m3: max finite 240, overflow -> inf.
# Keep scaled maxima comfortably below 240 (|u|<~0.75, |ev|<~30, ek<~12).
U_SCALE = 256.0     # fp8 scale for u = exp(pos_bias) - 1
KV_SCALE = 4.0      # fp8 scale for [ev|ek]
INV_SCALE = 1.0 / (U_SCALE * KV_SCALE)


def _install_axon_ntff_shim():
    """Make run_bass_kernel_spmd(trace=True) work when the image's antenv
    lacks axon_hooks (the hook degrades tracing otherwise).  No-op when a
    real antenv.axon_hooks is importable."""
    import sys
    import types

    try:
        import antenv.axon_hooks  # noqa: F401
        return
    except ImportError:
        pass
    try:
        from trn_agent_boot.trn_boot import _ntff_profile_via_ctypes
        hook = _ntff_profile_via_ctypes("/opt/axon/libaxon_pjrt.so")
    except Exception:
        hook = None
    mod = types.ModuleType("antenv.axon_hooks")
    mod.get_axon_ntff_profile_hook = lambda: hook
    mod.set_axon_ntff_profile_hook = lambda h: None
    sys.modules["antenv.axon_hooks"] = mod

    import concourse.bass_utils as bass_utils
    _orig_upload = bass_utils.upload_artifacts

    def _safe_upload(tmpdir):
        try:
            return _orig_upload(tmpdir)
        except Exception:
            return tmpdir

    bass_utils.upload_artifacts = _safe_upload


def build_aft(B=2, N=2048, D=512, n_cores=8, use_bias=False):
    NT = N // P          # row tiles per batch (t / j / i tiles)
    DB = D // P          # d_model blocks of 128 (contraction for projections)
    QKV = 3 * D
    C2 = 2 * B * D       # stage-2 psum width: [num_b0|den_b0|num_b1|den_b1]
    XW = 4 * P           # x DMA batching: four t-tiles per transfer (2KB runs)
    Exp = mybir.ActivationFunctionType.Exp
    Ident = mybir.ActivationFunctionType.Identity
    Alu = mybir.AluOpType
    DR = mybir.MatmulPerfMode.DoubleRow
    F32R = mybir.dt.float32r

    nc = bacc.Bacc("TRN2", target_bir_lowering=False, debug=False,
                   num_devices=n_cores)

    xT_e = nc.dram_tensor("xT", [B, D, N], F32, kind="ExternalInput")
    w_e = nc.dram_tensor("wvkq", [D, QKV], F32, kind="ExternalInput")
    pbT_e = nc.dram_tensor("pbT", [N, N], F32, kind="ExternalInput")
    if use_bias:
        b_e = nc.dram_tensor("bvkq", [1, QKV], F32, kind="ExternalInput")
    out_e = nc.dram_tensor("out", [B, N, D], F32, kind="ExternalOutput")

    with tile.TileContext(nc) as tc, ExitStack() as ctx:
        persist = ctx.enter_context(tc.tile_pool(name="persist", bufs=1))
        psp = ctx.enter_context(tc.tile_pool(name="psum", bufs=2, space="PSUM"))

        # ---- persistent SBUF tensors ----
        u8_sb = persist.tile([P, NT, N], F8)             # 512*(exp(pbT)-1)
        ekv_sb = persist.tile([P, NT, 2 * B * D], F8)    # 8*[ev|ek] per batch
        q_sb = persist.tile([P, B * NT, D], BF16)        # exp(q)
        acc_sb = persist.tile([P, 2 * B * D], F32)       # colsum accumulator
        cs_sb = persist.tile([P, 2 * B * D], F32)        # all-reduced colsums
        cneg = persist.tile([P, 1], F32)                 # bias AP: -U_SCALE
        nc.gpsimd.memset(cneg[:], -U_SCALE)

        with ExitStack() as s1:
            wpool = s1.enter_context(tc.tile_pool(name="wpool", bufs=1))
            stage = s1.enter_context(tc.tile_pool(name="stage", bufs=2))
            xstage = s1.enter_context(tc.tile_pool(name="xstage", bufs=3))
            # deep kv pool: the colsum-accumulate chains drain a few tiles
            # behind the ACT/DVE epilogue ops and must not backpressure them
            kvpool = s1.enter_context(tc.tile_pool(name="kvpool", bufs=6))
            ebpool = s1.enter_context(tc.tile_pool(name="ebpool", bufs=2))

            # ---- PE warmup ----
            # The PE clock-gate (HAM) starts at 1.2GHz and releases to
            # 2.4GHz only after ~3.4us of sustained activity.  The first
            # ~14us of the kernel are DMA-bound with an idle PE, so issue
            # throwaway matmuls on a memset tile to warm the clock before
            # the first real projection arrives.
            wa = wpool.tile([P, 64], BF16)
            nc.gpsimd.memset(wa[:], 0.0)
            nc.gpsimd.memset(acc_sb[:], 0.0)
            wps = psp.tile([P, C2], F32, tag="ps", name="wps")
            for w_i in range(125):
                nc.tensor.matmul(wps[0:64, 0:64], wa[:, 0:64], wa[:],
                                 start=(w_i == 0), stop=(w_i == 124))

            # ---- weights: DMA f32 per d-block + cast to bf16 ----
            # db0 is split [v | kq] so the very first matmul's weights (v,
            # db0) arrive with a minimal transfer instead of waiting for
            # 3MB of weight DMA to drain.
            w_sb = wpool.tile([P, DB, QKV], BF16)        # rhs for projections
            w_r = w_e.ap().rearrange("(db p) c -> db p c", p=P)
            IOW = max(N // 2, QKV)
            w_st = stage.tile([P, IOW], F32, tag="io", name="w_st")
            nc.sync.dma_start(w_st[:, :D], w_r[0][:, :D])
            nc.vector.tensor_copy(w_sb[:, 0, :D], w_st[:, :D])
            w_st = stage.tile([P, IOW], F32, tag="io", name="w_st")
            nc.sync.dma_start(w_st[:, :QKV - D], w_r[0][:, D:])
            nc.vector.tensor_copy(w_sb[:, 0, D:], w_st[:, :QKV - D])
            for db in range(1, DB):
                w_st = stage.tile([P, IOW], F32, tag="io", name="w_st")
                nc.sync.dma_start(w_st[:, :QKV], w_r[db])
                nc.vector.tensor_copy(w_sb[:, db, :], w_st[:, :QKV])
            if use_bias:
                b_st = stage.tile([1, QKV], F32, tag="bst")
                nc.sync.dma_start(b_st[:], b_e.ap())
                bias_sb = wpool.tile([1, QKV], BF16)
                nc.vector.tensor_copy(bias_sb[:], b_st[:])
                ones_sb = wpool.tile([1, P], BF16)
                nc.vector.memset(ones_sb[:], 1.0)

            # ---- stage 1: projections v/k/q + exp epilogue ----
            # pos-bias blocks are paced into the loop (left column half
            # only, one 512KB block per two t-tiles) so the pbT stream
            # doesn't starve the x DMAs feeding the projections.
            xT_r = xT_e.ap().rearrange("b (db p) n -> b p db n", p=P)
            XT = XW // P         # t-tiles per x transfer
            # chunk the t axis: small leading chunks on batch 0 so the first
            # matmul's x tile doesn't wait behind a 2MB transfer
            def x_chunks(b, NT=NT, XT=XT):
                sizes = [1, 1, 1, 1] if b == 0 else []
                while sum(sizes) < NT:
                    sizes.append(min(XT, NT - sum(sizes)))
                return sizes

            # x is cast bf16 at CHUNK granularity, one contiguous DVE op
            # per transfer (a per-tile strided cast measured ~2x slower
            # per element); the cast is emitted right after the chunk DMA
            # so it runs while the previous chunk's matmuls execute.
            chunk_info = []          # (b, t0, cw) per transfer
            tile_chunk = {}          # global tile s -> (chunk idx, tloc)
            for b in range(B):
                t0 = 0
                for cw in x_chunks(b):
                    for u in range(cw):
                        tile_chunk[b * NT + t0 + u] = (len(chunk_info), u)
                    chunk_info.append((b, t0, cw))
                    t0 += cw

            x_bf_c = {}              # chunk idx -> bf16 chunk tile

            def emit_chunk_dma(cidx):
                cb, ct0, cw = chunk_info[cidx]
                x_st = xstage.tile([P, DB, XW], F32, tag="xst", bufs=2,
                                   name="x_st")
                nc.sync.dma_start(
                    x_st[:, :, :cw * P],
                    xT_r[cb, :, :, ct0 * P:(ct0 + cw) * P])
                x_bf = xstage.tile([P, DB, XW], BF16, tag="xbf", bufs=2,
                                   name="x_bf")
                nc.vector.tensor_copy(x_bf[:, :, :cw * P],
                                      x_st[:, :, :cw * P])
                x_bf_c[cidx] = x_bf

            def emit_pb_left(jb):
                # left column half of the shifted pos-bias transform:
                # u8 = exp(pbT)*256 - 256 in fp8, both steps on the scalar
                # engine (keeps the DVE free for the x casts / ekv)
                pb_st = stage.tile([P, IOW], F32, tag="io", name="pb_st")
                nc.sync.dma_start(pb_st[:, :N // 2],
                                  pbT_e.ap()[jb * P:(jb + 1) * P, :N // 2])
                ebf = ebpool.tile([P, N // 2], F32, tag="ebf")
                nc.scalar.activation(ebf[:], pb_st[:, :N // 2], Exp)
                nc.scalar.activation(u8_sb[:, jb, :N // 2], ebf[:], Ident,
                                     bias=cneg[:], scale=U_SCALE)

            emit_chunk_dma(0)
            emit_chunk_dma(1)
            step = 0
            for b in range(B):
                for t in range(NT):
                    s = b * NT + t
                    cidx, tloc = tile_chunk[s]
                    # keep TWO chunks of DMA lead: the x stream shares the
                    # DMA queues with the pos-bias blocks and needs slack
                    if tloc == 0 and cidx + 2 < len(chunk_info):
                        emit_chunk_dma(cidx + 2)
                    x_bf = x_bf_c[cidx]

                    ps = psp.tile([P, C2], F32, tag="ps")
                    for db in range(DB):
                        for n3 in range(3):   # [v|k|q]
                            nc.tensor.matmul(
                                ps[:, n3 * D:(n3 + 1) * D],
                                x_bf[:, db, tloc * P:(tloc + 1) * P],
                                w_sb[:, db, n3 * D:(n3 + 1) * D],
                                start=(db == 0),
                                stop=(db == DB - 1 and not use_bias))
                    if use_bias:
                        for n3 in range(3):
                            nc.tensor.matmul(
                                ps[:, n3 * D:(n3 + 1) * D],
                                ones_sb[:, :],
                                bias_sb[:, n3 * D:(n3 + 1) * D],
                                start=False, stop=True)

                    col = b * 2 * D
                    # kv = [ev | ek] in bf16 (feeds the f32 colsum -- bf16
                    # element rounding costs only ~0.1% there -- and the
                    # fp8 cast; 16-bit keeps the DVE/gpsimd ops at 2x rate)
                    kv = kvpool.tile([P, 2 * D], BF16, tag="kv")
                    nc.scalar.activation(kv[:, D:2 * D], ps[:, D:2 * D], Exp)
                    nc.vector.tensor_mul(kv[:, 0:D], kv[:, D:2 * D],
                                         ps[:, 0:D])
                    # em = exp(-q) straight from PSUM on the scalar engine;
                    # the epilogue uses sigmoid(q)*num/den =
                    # num / (den * (1 + exp(-q)))
                    nc.scalar.activation(q_sb[:, b * NT + t, :],
                                         ps[:, 2 * D:3 * D], Exp, scale=-1.0)
                    # fp8 cast (scaled) in one DVE op
                    nc.vector.tensor_scalar_mul(
                        ekv_sb[:, t, col:col + 2 * D], kv[:], KV_SCALE)
                    # f32 colsum accumulation: a serial per-batch chain.
                    # b0's chain runs on gpsimd; b1's runs on the (faster)
                    # DVE so it finishes with stage 1 and its all-reduce
                    # result is ready before stage-2 i=2 recycles PSUM.
                    acc_eng = nc.gpsimd if b == 0 else nc.vector
                    acc_eng.tensor_add(acc_sb[:, col:col + 2 * D],
                                       acc_sb[:, col:col + 2 * D], kv[:])

                    # pace pos-bias blocks, LEFT COLUMN HALF only: stage-2
                    # i-tile i reads columns i*128:(i+1)*128 of each block,
                    # so the right half isn't needed until i=NT/2 -- it is
                    # loaded during stage 2 where DMA is otherwise idle.
                    # One 512KB half-block per two t-tiles covers all NT
                    # blocks within stage 1 without crowding the x DMAs.
                    if step >= 7 and step % 2 == 1:
                        jb = (step - 7) // 2
                        if jb < NT:
                            emit_pb_left(jb)
                    step += 1

                # batch b's colsum: reduce acc over partitions and
                # broadcast to all partitions, on the (otherwise idle)
                # gpsimd engine.  b0's runs overlapped with b1's stage 1.
                col = b * 2 * D
                nc.gpsimd.partition_all_reduce(
                    cs_sb[:, col:col + 2 * D], acc_sb[:, col:col + 2 * D],
                    P, ReduceOp.add)

            # left-half remainder: 512KB blocks load fast at the stage-2
            # head and are consumed last by i=0's ascending jb sweep
            for jb in range(max(0, (step - 7 + 1) // 2), NT):
                emit_pb_left(jb)

        # ---- stage 2: num/den contraction over j + epilogue ----
        epi = ctx.enter_context(tc.tile_pool(name="epi", bufs=3))
        pb1p = ctx.enter_context(tc.tile_pool(name="pb1p", bufs=2))

        # right-half pos-bias pacing: 3 blocks per early i-tile so all 16
        # are transformed well before i = NT/2 reads them
        pbr_sched = {}
        jbr_next = 0
        for i in range(NT):
            take = min(3, NT - jbr_next)
            pbr_sched[i] = list(range(jbr_next, jbr_next + take))
            jbr_next += take

        for i in range(NT):
            ps = psp.tile([P, C2], F32, tag="ps")
            for jb2 in range(NT // 2):
                lhsT = u8_sb[:, 2 * jb2:2 * jb2 + 2, i * P:(i + 1) * P]
                for n4 in range(2 * B):
                    nc.tensor.matmul(
                        ps[:, n4 * D:(n4 + 1) * D],
                        lhsT,
                        ekv_sb[:, 2 * jb2:2 * jb2 + 2,
                               n4 * D:(n4 + 1) * D],
                        start=(jb2 == 0), stop=(jb2 == NT // 2 - 1),
                        perf_mode=DR)

            # right column halves of the shifted pos-bias: exp on the
            # scalar engine; the shift-and-quantize alternates DVE / ACT
            # so neither engine eats the full 3-blocks-per-i-tile burst
            for jbr in pbr_sched[i]:
                pb1 = pb1p.tile([P, N // 2], F32, tag="pb1")
                nc.sync.dma_start(
                    pb1[:], pbT_e.ap()[jbr * P:(jbr + 1) * P, N // 2:])
                eb1 = pb1p.tile([P, N // 2], BF16, tag="eb1")
                nc.scalar.activation(eb1[:], pb1[:], Exp)
                if jbr % 2 == 0:
                    nc.vector.tensor_scalar(u8_sb[:, jbr, N // 2:], eb1[:],
                                            1.0, U_SCALE,
                                            Alu.subtract, Alu.mult)
                else:
                    nc.scalar.activation(u8_sb[:, jbr, N // 2:], eb1[:],
                                         Ident, bias=cneg[:], scale=U_SCALE)

            o = epi.tile([P, B, D], F32, tag="o")
            for b in range(B):
                col = b * 2 * D
                nu = ps[:, col:col + D]
                de = ps[:, col + D:col + 2 * D]
                emp1 = q_sb[:, b * NT + i, :]
                # emp1 = 1 + exp(-q), in place on the stage-2 scalar engine
                nc.scalar.activation(emp1, emp1, Ident, bias=1.0)
                # num/den = psum/1024 + colsum   (the exact shifted term)
                num = epi.tile([P, D], F32, tag="num")
                nc.vector.scalar_tensor_tensor(
                    num[:], nu, INV_SCALE, cs_sb[:, col:col + D],
                    Alu.mult, Alu.add)
                den = epi.tile([P, D], F32, tag="den")
                nc.vector.scalar_tensor_tensor(
                    den[:], de, INV_SCALE, cs_sb[:, col + D:col + 2 * D],
                    Alu.mult, Alu.add)
                # t1 = (1 + exp(-q)) * den, a plain multiply on the idle
                # stage-2 gpsimd (Pool supports tensor_tensor only); the
                # final i-tile keeps it on the DVE to shorten the tail
                t1 = epi.tile([P, D], F32, tag="t1")
                t1_eng = nc.vector if i == NT - 1 else nc.gpsimd
                t1_eng.tensor_mul(t1[:], emp1, den[:])
                r = epi.tile([P, D], F32, tag="r")
                nc.vector.reciprocal_approx_fast(r[:], t1[:])
                nc.vector.tensor_mul(o[:, b, :], num[:], r[:])
                if i == NT - 1:
                    # last tile: per-batch DMA so the b0 store overlaps the
                    # b1 epilogue instead of extending the kernel tail
                    nc.sync.dma_start(out_e.ap()[b, i * P:(i + 1) * P],
                                      o[:, b, :])
            if i < NT - 1:
                nc.sync.dma_start(
                    out_e.ap().rearrange("b n d -> n b d")[i * P:(i + 1) * P],
                    o[:])

    nc.compile()
    return nc


_NC_CACHE = {}


def _get_nc(use_bias):
    key = bool(use_bias)
    if key not in _NC_CACHE:
        _NC_CACHE[key] = build_aft(B=BATCH // N_CORES, N=N, D=D_MODEL,
                                   n_cores=N_CORES, use_bias=key)
    return _NC_CACHE[key]


def kernel(x, Wq, bq, Wk, bk, Wv, bv, pos_bias):
    x = np.asarray(x, dtype=np.float32)
    Wq = np.asarray(Wq, dtype=np.float32)
    Wk = np.asarray(Wk, dtype=np.float32)
    Wv = np.asarray(Wv, dtype=np.float32)
    bq = np.asarray(bq, dtype=np.float32)
    bk = np.asarray(bk, dtype=np.float32)
    bv = np.asarray(bv, dtype=np.float32)
    pos_bias = np.asarray(pos_bias, dtype=np.float32)
    assert x.shape == (BATCH, N, D_MODEL)
    assert pos_bias.shape == (N, N)

    _install_axon_ntff_shim()

    use_bias = bool(np.any(bq) or np.any(bk) or np.any(bv))
    nc = _get_nc(use_bias)

    Bc = BATCH // N_CORES
    wvkq = np.concatenate([Wv, Wk, Wq], axis=1)           # [D, 3D]
    pbT = np.ascontiguousarray(pos_bias.T)                # [N, N]
    in_maps = []
    for c in range(N_CORES):
        im = {
            "xT": np.ascontiguousarray(
                x[c * Bc:(c + 1) * Bc].transpose(0, 2, 1)),
            "wvkq": wvkq,
            "pbT": pbT,
        }
        if use_bias:
            im["bvkq"] = np.concatenate([bv, bk, bq])[None, :]
        in_maps.append(im)

    res = run_bass_kernel_spmd(nc, in_maps, core_ids=list(range(N_CORES)))
    out = np.concatenate([res.results[c]["out"] for c in range(N_CORES)],
                         axis=0)
    return out.astype(np.float32, copy=False)


# revision 37
# speedup vs baseline: 1.3611x; 1.0021x over previous
"""AFT-Full (Attention Free Transformer, full position bias) on 8 TRN2
NeuronCores.

Problem (per reference.py):
    x [16, 2048, 512] f32, Wq/Wk/Wv [512, 512], bq/bk/bv [512],
    pos_bias [2048, 2048]
    q = x@Wq+bq; k = x@Wk+bk; v = x@Wv+bv
    out[b,i,d] = sigmoid(q)[b,i,d]
                 * sum_j exp(k+bias[i,j])*v / sum_j exp(k+bias[i,j])

Sharding: pure data-parallel over the batch (16 batches -> 2 per core).
Every core holds a replica of the weights and pos_bias; there is zero
cross-core communication.

Numerics / speed strategy:
  - Stage 1 (projections v/k/q) runs in bf16 on the TensorEngine.
  - Stage 2 (the [N,N] x [N,2BD] num/den contraction) runs in fp8e4
    with the DoubleRow perf mode (2 contraction rows per PE pass).
    Naive fp8 here costs ~3.6% output error because the output is a
    near-cancelling weighted mean of zero-mean v and per-term
    quantization noise passes straight through.  Instead we use the
    shifted decomposition
        eb = exp(pos_bias) = 1 + u,   u = exp(pos_bias) - 1
        num[i,d] = sum_j ev[j,d]  +  sum_j u[i,j] ev[j,d]
        den[i,d] = sum_j ek[j,d]  +  sum_j u[i,j] ek[j,d]
    The first (i-independent) colsum terms carry ~90% of the magnitude
    and are accumulated exactly in f32 on the sbuf side; only the small
    u-contraction runs in fp8 (u has RMS ~0.1 vs eb ~1.0), cutting the
    fp8 noise by ~10x (to ~0.4% total, vs the 2e-2 harness gate).
    u is scaled by 512 and [ev|ek] by 8 to keep fp8 values in the
    normal e4m3 range; the epilogue divides the PSUM result by 4096
    and adds the colsums back before the sigmoid gate.
  - sigmoid(q)*num/den = num*exp(q) / (den*(1+exp(q))), so the scalar
    engine only ever evaluates Exp.
"""

from contextlib import ExitStack

import numpy as np

import concourse.bacc as bacc
import concourse.mybir as mybir
import concourse.tile as tile
from concourse.bass_isa import ReduceOp
from concourse.bass_utils import run_bass_kernel_spmd

F32 = mybir.dt.float32
BF16 = mybir.dt.bfloat16
F8 = mybir.dt.float8e4
P = 128

N_CORES = 8
BATCH = 16
N = 2048
D_MODEL = 512

# mybir float8e4 is IEEE-style e4m3: max finite 240, overflow -> inf.
# Keep scaled maxima comfortably below 240 (|u|<~0.75, |ev|<~30, ek<~12).
U_SCALE = 256.0     # fp8 scale for u = exp(pos_bias) - 1
KV_SCALE = 4.0      # fp8 scale for [ev|ek]
INV_SCALE = 1.0 / (U_SCALE * KV_SCALE)


def _install_axon_ntff_shim():
    """Make run_bass_kernel_spmd(trace=True) work when the image's antenv
    lacks axon_hooks (the hook degrades tracing otherwise).  No-op when a
    real antenv.axon_hooks is importable."""
    import sys
    import types

    try:
        import antenv.axon_hooks  # noqa: F401
        return
    except ImportError:
        pass
    try:
        from trn_agent_boot.trn_boot import _ntff_profile_via_ctypes
        hook = _ntff_profile_via_ctypes("/opt/axon/libaxon_pjrt.so")
    except Exception:
        hook = None
    mod = types.ModuleType("antenv.axon_hooks")
    mod.get_axon_ntff_profile_hook = lambda: hook
    mod.set_axon_ntff_profile_hook = lambda h: None
    sys.modules["antenv.axon_hooks"] = mod

    import concourse.bass_utils as bass_utils
    _orig_upload = bass_utils.upload_artifacts

    def _safe_upload(tmpdir):
        try:
            return _orig_upload(tmpdir)
        except Exception:
            return tmpdir

    bass_utils.upload_artifacts = _safe_upload


def build_aft(B=2, N=2048, D=512, n_cores=8, use_bias=False):
    NT = N // P          # row tiles per batch (t / j / i tiles)
    DB = D // P          # d_model blocks of 128 (contraction for projections)
    QKV = 3 * D
    C2 = 2 * B * D       # stage-2 psum width: [num_b0|den_b0|num_b1|den_b1]
    XW = 4 * P           # x DMA batching: four t-tiles per transfer (2KB runs)
    Exp = mybir.ActivationFunctionType.Exp
    Ident = mybir.ActivationFunctionType.Identity
    Alu = mybir.AluOpType
    DR = mybir.MatmulPerfMode.DoubleRow
    F32R = mybir.dt.float32r

    nc = bacc.Bacc("TRN2", target_bir_lowering=False, debug=False,
                   num_devices=n_cores)

    xT_e = nc.dram_tensor("xT", [B, D, N], F32, kind="ExternalInput")
    w_e = nc.dram_tensor("wvkq", [D, QKV], F32, kind="ExternalInput")
    pbT_e = nc.dram_tensor("pbT", [N, N], F32, kind="ExternalInput")
    if use_bias:
        b_e = nc.dram_tensor("bvkq", [1, QKV], F32, kind="ExternalInput")
    out_e = nc.dram_tensor("out", [B, N, D], F32, kind="ExternalOutput")

    with tile.TileContext(nc) as tc, ExitStack() as ctx:
        persist = ctx.enter_context(tc.tile_pool(name="persist", bufs=1))
        psp = ctx.enter_context(tc.tile_pool(name="psum", bufs=2, space="PSUM"))

        # ---- persistent SBUF tensors ----
        u8_sb = persist.tile([P, NT, N], F8)             # 512*(exp(pbT)-1)
        ekv_sb = persist.tile([P, NT, 2 * B * D], F8)    # 8*[ev|ek] per batch
        q_sb = persist.tile([P, B * NT, D], BF16)        # exp(q)
        acc_sb = persist.tile([P, 2 * B * D], F32)       # colsum accumulator
        cs_sb = persist.tile([P, 2 * B * D], F32)        # all-reduced colsums
        cneg = persist.tile([P, 1], F32)                 # bias AP: -U_SCALE
        nc.gpsimd.memset(cneg[:], -U_SCALE)

        with ExitStack() as s1:
            wpool = s1.enter_context(tc.tile_pool(name="wpool", bufs=1))
            stage = s1.enter_context(tc.tile_pool(name="stage", bufs=2))
            xstage = s1.enter_context(tc.tile_pool(name="xstage", bufs=3))
            # deep kv pool: the colsum-accumulate chains drain a few tiles
            # behind the ACT/DVE epilogue ops and must not backpressure them
            kvpool = s1.enter_context(tc.tile_pool(name="kvpool", bufs=6))
            ebpool = s1.enter_context(tc.tile_pool(name="ebpool", bufs=2))

            # ---- PE warmup ----
            # The PE clock-gate (HAM) starts at 1.2GHz and releases to
            # 2.4GHz only after ~3.4us of sustained activity.  The first
            # ~14us of the kernel are DMA-bound with an idle PE, so issue
            # throwaway matmuls on a memset tile to warm the clock before
            # the first real projection arrives.
            wa = wpool.tile([P, 64], BF16)
            nc.gpsimd.memset(wa[:], 0.0)
            nc.gpsimd.memset(acc_sb[:], 0.0)
            wps = psp.tile([P, C2], F32, tag="ps", name="wps")
            for w_i in range(125):
                nc.tensor.matmul(wps[0:64, 0:64], wa[:, 0:64], wa[:],
                                 start=(w_i == 0), stop=(w_i == 124))

            # ---- weights: DMA f32 per d-block + cast to bf16 ----
            # db0 is split [v | kq] so the very first matmul's weights (v,
            # db0) arrive with a minimal transfer instead of waiting for
            # 3MB of weight DMA to drain.
            w_sb = wpool.tile([P, DB, QKV], BF16)        # rhs for projections
            w_r = w_e.ap().rearrange("(db p) c -> db p c", p=P)
            IOW = max(N // 2, QKV)
            w_st = stage.tile([P, IOW], F32, tag="io", name="w_st")
            nc.sync.dma_start(w_st[:, :D], w_r[0][:, :D])
            nc.vector.tensor_copy(w_sb[:, 0, :D], w_st[:, :D])
            w_st = stage.tile([P, IOW], F32, tag="io", name="w_st")
            nc.sync.dma_start(w_st[:, :QKV - D], w_r[0][:, D:])
            nc.vector.tensor_copy(w_sb[:, 0, D:], w_st[:, :QKV - D])
            for db in range(1, DB):
                w_st = stage.tile([P, IOW], F32, tag="io", name="w_st")
                nc.sync.dma_start(w_st[:, :QKV], w_r[db])
                nc.vector.tensor_copy(w_sb[:, db, :], w_st[:, :QKV])
            if use_bias:
                b_st = stage.tile([1, QKV], F32, tag="bst")
                nc.sync.dma_start(b_st[:], b_e.ap())
                bias_sb = wpool.tile([1, QKV], BF16)
                nc.vector.tensor_copy(bias_sb[:], b_st[:])
                ones_sb = wpool.tile([1, P], BF16)
                nc.vector.memset(ones_sb[:], 1.0)

            # ---- stage 1: projections v/k/q + exp epilogue ----
            # pos-bias blocks are paced into the loop (left column half
            # only, one 512KB block per two t-tiles) so the pbT stream
            # doesn't starve the x DMAs feeding the projections.
            xT_r = xT_e.ap().rearrange("b (db p) n -> b p db n", p=P)
            XT = XW // P         # t-tiles per x transfer
            # chunk the t axis: small leading chunks on batch 0 so the first
            # matmul's x tile doesn't wait behind a 2MB transfer
            def x_chunks(b, NT=NT, XT=XT):
                sizes = [1, 1, 1, 1] if b == 0 else []
                while sum(sizes) < NT:
                    sizes.append(min(XT, NT - sum(sizes)))
                return sizes

            # x is cast bf16 at CHUNK granularity, one contiguous DVE op
            # per transfer (a per-tile strided cast measured ~2x slower
            # per element); the cast is emitted right after the chunk DMA
            # so it runs while the previous chunk's matmuls execute.
            chunk_info = []          # (b, t0, cw) per transfer
            tile_chunk = {}          # global tile s -> (chunk idx, tloc)
            for b in range(B):
                t0 = 0
                for cw in x_chunks(b):
                    for u in range(cw):
                        tile_chunk[b * NT + t0 + u] = (len(chunk_info), u)
                    chunk_info.append((b, t0, cw))
                    t0 += cw

            x_st_c = {}              # chunk idx -> staged f32 tile
            x_bf_c = {}              # chunk idx -> bf16 chunk tile

            def emit_chunk_dma(cidx):
                if cidx >= len(chunk_info) or cidx in x_st_c:
                    return
                cb, ct0, cw = chunk_info[cidx]
                x_st = xstage.tile([P, DB, XW], F32, tag="xst", bufs=3,
                                   name="x_st")
                nc.sync.dma_start(
                    x_st[:, :, :cw * P],
                    xT_r[cb, :, :, ct0 * P:(ct0 + cw) * P])
                x_st_c[cidx] = x_st

            def emit_chunk_cast(cidx):
                # emitted only once the chunk's DMA has had time to land,
                # so this op never stalls the DVE FIFO head
                if cidx >= len(chunk_info) or cidx in x_bf_c:
                    return
                cw = chunk_info[cidx][2]
                x_bf = xstage.tile([P, DB, XW], BF16, tag="xbf", bufs=2,
                                   name="x_bf")
                nc.vector.tensor_copy(x_bf[:, :, :cw * P],
                                      x_st_c[cidx][:, :, :cw * P])
                x_bf_c[cidx] = x_bf

            def emit_pb_left(jb):
                # left column half of the shifted pos-bias transform:
                # u8 = exp(pbT)*256 - 256 in fp8, both steps on the scalar
                # engine (keeps the DVE free for the x casts / ekv)
                pb_st = stage.tile([P, IOW], F32, tag="io", name="pb_st")
                nc.sync.dma_start(pb_st[:, :N // 2],
                                  pbT_e.ap()[jb * P:(jb + 1) * P, :N // 2])
                ebf = ebpool.tile([P, N // 2], F32, tag="ebf")
                nc.scalar.activation(ebf[:], pb_st[:, :N // 2], Exp)
                nc.scalar.activation(u8_sb[:, jb, :N // 2], ebf[:], Ident,
                                     bias=cneg[:], scale=U_SCALE)

            kv_t = {}                # tile s -> (kv tile, batch)

            def emit_epi_tail(s):
                # DEFERRED one tile: the fp8 cast + colsum add for tile s
                # are emitted during tile s+1, so in the DVE FIFO the next
                # tile's ev-multiply sits directly behind ready-to-run work
                # instead of behind ops still waiting on DMA.
                kv, kb = kv_t.pop(s)
                col = kb * 2 * D
                nc.vector.tensor_scalar_mul(
                    ekv_sb[:, s - kb * NT, col:col + 2 * D], kv[:], KV_SCALE)
                # f32 colsum accumulation: a serial per-batch chain.  b0's
                # runs on gpsimd; b1's on the (faster) DVE so it finishes
                # with stage 1 and its all-reduce result is ready before
                # stage-2 i=2 recycles PSUM.
                acc_eng = nc.gpsimd if kb == 0 else nc.vector
                acc_eng.tensor_add(acc_sb[:, col:col + 2 * D],
                                   acc_sb[:, col:col + 2 * D], kv[:])

            def emit_allreduce(b):
                # reduce acc over partitions and broadcast to all
                # partitions, on the gpsimd engine.  b0's runs overlapped
                # with b1's stage 1.
                col = b * 2 * D
                nc.gpsimd.partition_all_reduce(
                    cs_sb[:, col:col + 2 * D], acc_sb[:, col:col + 2 * D],
                    P, ReduceOp.add)

            emit_chunk_dma(0)
            emit_chunk_dma(1)
            emit_chunk_cast(0)
            step = 0
            for b in range(B):
                for t in range(NT):
                    s = b * NT + t
                    cidx, tloc = tile_chunk[s]
                    cw = chunk_info[cidx][2]
                    if s > 0:
                        emit_epi_tail(s - 1)
                    if s == NT + 1:
                        emit_allreduce(0)
                    # keep TWO chunks of DMA lead (the x stream shares the
                    # DMA queues with the pos-bias blocks); cast the NEXT
                    # chunk late in the current one, once its data landed
                    if tloc == 0:
                        emit_chunk_dma(cidx + 2)
                    if tloc == max(cw - 2, 0):
                        emit_chunk_cast(cidx + 1)
                    x_bf = x_bf_c[cidx]

                    ps = psp.tile([P, C2], F32, tag="ps")
                    for db in range(DB):
                        for n3 in range(3):   # [v|k|q]
                            nc.tensor.matmul(
                                ps[:, n3 * D:(n3 + 1) * D],
                                x_bf[:, db, tloc * P:(tloc + 1) * P],
                                w_sb[:, db, n3 * D:(n3 + 1) * D],
                                start=(db == 0),
                                stop=(db == DB - 1 and not use_bias))
                    if use_bias:
                        for n3 in range(3):
                            nc.tensor.matmul(
                                ps[:, n3 * D:(n3 + 1) * D],
                                ones_sb[:, :],
                                bias_sb[:, n3 * D:(n3 + 1) * D],
                                start=False, stop=True)

                    # kv = [ev | ek] in bf16 (feeds the f32 colsum -- bf16
                    # element rounding costs only ~0.1% there -- and the
                    # fp8 cast; 16-bit keeps the DVE/gpsimd ops at 2x rate)
                    kv = kvpool.tile([P, 2 * D], BF16, tag="kv")
                    nc.scalar.activation(kv[:, D:2 * D], ps[:, D:2 * D], Exp)
                    # em = exp(-q) straight from PSUM on the scalar engine;
                    # the epilogue uses sigmoid(q)*num/den =
                    # num / (den * (1 + exp(-q)))
                    nc.scalar.activation(q_sb[:, b * NT + t, :],
                                         ps[:, 2 * D:3 * D], Exp, scale=-1.0)
                    nc.vector.tensor_mul(kv[:, 0:D], kv[:, D:2 * D],
                                         ps[:, 0:D])
                    kv_t[s] = (kv, b)

                    # pace pos-bias blocks, LEFT COLUMN HALF only: stage-2
                    # i-tile i reads columns i*128:(i+1)*128 of each block,
                    # so the right half isn't needed until i=NT/2 -- it is
                    # loaded during stage 2 where DMA is otherwise idle.
                    # One 512KB half-block per two t-tiles covers all NT
                    # blocks within stage 1 without crowding the x DMAs.
                    if step >= 7 and step % 2 == 1:
                        jb = (step - 7) // 2
                        if jb < NT:
                            emit_pb_left(jb)
                    step += 1

            emit_epi_tail(B * NT - 1)
            emit_allreduce(1)

            # left-half remainder: 512KB blocks load fast at the stage-2
            # head and are consumed last by i=0's ascending jb sweep
            for jb in range(max(0, (step - 7 + 1) // 2), NT):
                emit_pb_left(jb)

        # ---- stage 2: num/den contraction over j + epilogue ----
        epi = ctx.enter_context(tc.tile_pool(name="epi", bufs=3))
        pb1p = ctx.enter_context(tc.tile_pool(name="pb1p", bufs=4))

        # right-half pos-bias pacing: 3 blocks per early i-tile so all 16
        # are transformed well before i = NT/2 reads them
        pbr_sched = {}
        jbr_next = 0
        for i in range(NT):
            take = min(3, NT - jbr_next)
            pbr_sched[i] = list(range(jbr_next, jbr_next + take))
            jbr_next += take

        for i in range(NT):
            ps = psp.tile([P, C2], F32, tag="ps")
            for jb2 in range(NT // 2):
                lhsT = u8_sb[:, 2 * jb2:2 * jb2 + 2, i * P:(i + 1) * P]
                for n4 in range(2 * B):
                    nc.tensor.matmul(
                        ps[:, n4 * D:(n4 + 1) * D],
                        lhsT,
                        ekv_sb[:, 2 * jb2:2 * jb2 + 2,
                               n4 * D:(n4 + 1) * D],
                        start=(jb2 == 0), stop=(jb2 == NT // 2 - 1),
                        perf_mode=DR)

            # right column halves of the shifted pos-bias: exp on the
            # scalar engine; the shift-and-quantize alternates DVE / ACT
            # so neither engine eats the full 3-blocks-per-i-tile burst
            for jbr in pbr_sched[i]:
                pb1 = pb1p.tile([P, N // 2], F32, tag="pb1")
                nc.sync.dma_start(
                    pb1[:], pbT_e.ap()[jbr * P:(jbr + 1) * P, N // 2:])
                eb1 = pb1p.tile([P, N // 2], BF16, tag="eb1")
                nc.scalar.activation(eb1[:], pb1[:], Exp)
                if jbr % 2 == 0:
                    nc.vector.tensor_scalar(u8_sb[:, jbr, N // 2:], eb1[:],
                                            1.0, U_SCALE,
                                            Alu.subtract, Alu.mult)
                else:
                    nc.scalar.activation(u8_sb[:, jbr, N // 2:], eb1[:],
                                         Ident, bias=cneg[:], scale=U_SCALE)

            o = epi.tile([P, B, D], F32, tag="o")
            for b in range(B):
                col = b * 2 * D
                nu = ps[:, col:col + D]
                de = ps[:, col + D:col + 2 * D]
                emp1 = q_sb[:, b * NT + i, :]
                # emp1 = 1 + exp(-q), in place on the stage-2 scalar engine
                nc.scalar.activation(emp1, emp1, Ident, bias=1.0)
                # num/den = psum/1024 + colsum   (the exact shifted term)
                num = epi.tile([P, D], F32, tag="num")
                nc.vector.scalar_tensor_tensor(
                    num[:], nu, INV_SCALE, cs_sb[:, col:col + D],
                    Alu.mult, Alu.add)
                den = epi.tile([P, D], F32, tag="den")
                nc.vector.scalar_tensor_tensor(
                    den[:], de, INV_SCALE, cs_sb[:, col + D:col + 2 * D],
                    Alu.mult, Alu.add)
                # t1 = (1 + exp(-q)) * den, a plain multiply on the idle
                # stage-2 gpsimd (Pool supports tensor_tensor only); the
                # final i-tile keeps it on the DVE to shorten the tail
                t1 = epi.tile([P, D], F32, tag="t1")
                t1_eng = nc.vector if i == NT - 1 else nc.gpsimd
                t1_eng.tensor_mul(t1[:], emp1, den[:])
                r = epi.tile([P, D], F32, tag="r")
                nc.vector.reciprocal_approx_fast(r[:], t1[:])
                nc.vector.tensor_mul(o[:, b, :], num[:], r[:])
                if i == NT - 1:
                    # last tile: per-batch DMA so the b0 store overlaps the
                    # b1 epilogue instead of extending the kernel tail
                    nc.sync.dma_start(out_e.ap()[b, i * P:(i + 1) * P],
                                      o[:, b, :])
            if i < NT - 1:
                nc.sync.dma_start(
                    out_e.ap().rearrange("b n d -> n b d")[i * P:(i + 1) * P],
                    o[:])

    nc.compile()
    return nc


_NC_CACHE = {}


def _get_nc(use_bias):
    key = bool(use_bias)
    if key not in _NC_CACHE:
        _NC_CACHE[key] = build_aft(B=BATCH // N_CORES, N=N, D=D_MODEL,
                                   n_cores=N_CORES, use_bias=key)
    return _NC_CACHE[key]


def kernel(x, Wq, bq, Wk, bk, Wv, bv, pos_bias):
    x = np.asarray(x, dtype=np.float32)
    Wq = np.asarray(Wq, dtype=np.float32)
    Wk = np.asarray(Wk, dtype=np.float32)
    Wv = np.asarray(Wv, dtype=np.float32)
    bq = np.asarray(bq, dtype=np.float32)
    bk = np.asarray(bk, dtype=np.float32)
    bv = np.asarray(bv, dtype=np.float32)
    pos_bias = np.asarray(pos_bias, dtype=np.float32)
    assert x.shape == (BATCH, N, D_MODEL)
    assert pos_bias.shape == (N, N)

    _install_axon_ntff_shim()

    use_bias = bool(np.any(bq) or np.any(bk) or np.any(bv))
    nc = _get_nc(use_bias)

    Bc = BATCH // N_CORES
    wvkq = np.concatenate([Wv, Wk, Wq], axis=1)           # [D, 3D]
    pbT = np.ascontiguousarray(pos_bias.T)                # [N, N]
    in_maps = []
    for c in range(N_CORES):
        im = {
            "xT": np.ascontiguousarray(
                x[c * Bc:(c + 1) * Bc].transpose(0, 2, 1)),
            "wvkq": wvkq,
            "pbT": pbT,
        }
        if use_bias:
            im["bvkq"] = np.concatenate([bv, bk, bq])[None, :]
        in_maps.append(im)

    res = run_bass_kernel_spmd(nc, in_maps, core_ids=list(range(N_CORES)))
    out = np.concatenate([res.results[c]["out"] for c in range(N_CORES)],
                         axis=0)
    return out.astype(np.float32, copy=False)
